# revision 1
# baseline (speedup 1.0000x reference)
"""Differential attention (two-softmax diff + GroupNorm) on 8 TRN2 cores.

Sharding: 16 heads / 8 cores = 2 heads per core (head-parallel, no
collectives). GroupNorm stats are per-(batch, head) so each core is fully
independent.

Device layout choices (host prepares everything):
  - Q, K per head are host-transposed to [128(d), 2048(s)] fp16: partitions
    0-63 hold half-1 (q1/k1), partitions 64-127 hold half-2. QK^T then
    contracts over the partition dim directly, producing transposed score
    blocks S^T[key, query] in PSUM (fp32). The two 64-contraction QK
    matmuls issue back-to-back and the PE runs them concurrently on row
    groups h0/h64.
  - V per head is prefixed with a ones column (V' = [1 | V], 65 cols, fp16)
    and pre-arranged into the SBUF image [128(key of block), 16*65]: the PV
    matmul (lhsT = V'[kblk], rhs = exp(S^T)[kblk]) then yields the softmax
    denominator on partition 0 and the numerator on partitions 1-64 in one
    accumulation group. BOTH halves share the same V' stationary (w1*V and
    lam*w2*V use the same V); lam is applied in the per-chunk epilogue via
    the fused LN_BWD_DX DVE op (out = t1 - lam*t2).
  - Output stays in [d, q] layout on device; the host transposes it back.

fp16 is used on every matmul path: it streams through the PE at ~1
cycle/column with the same 10-bit mantissa class as tf32. exp() runs on
ScalarE straight out of PSUM, writing fp16.

Main loop per (head, 512-query chunk): 16 key blocks of
  QK matmul pair -> exp on ScalarE (PSUM -> SBUF) -> PV pair accumulate,
then a per-chunk epilogue slice (denominator broadcast on GpSimd, divide +
combine + bn_stats on DVE) that hides under later chunks' main loop.
ScalarE's exp stream is the pacing engine (~1.1us per key block); the PE
fits just beneath it, so no warm-up spinner is used (the HAM clock gate
settles by itself and junk matmuls only delay the first real block).

Tail: rstd = sqrt(reciprocal_approx_fast(var+eps)) with the Sqrt act
table pre-loaded via a dummy activation right after the last exp, and the
final affine + output DMA run in 4 interleaved pieces.
"""

import math

import numpy as np

B, H, S, D = 1, 16, 2048, 64
N_CORES = 8
HPC = H // N_CORES  # heads per core
QC = 512            # query-chunk width (PSUM bank budget)
N_QC = S // QC
KB = S // 128       # key blocks of 128
LAMBDA_INIT = 0.8
EPS = 1e-5
SCALE = 1.0 / math.sqrt(D)
N_WARMUP_MM = 24

_CACHE = {}


def _build_nc():
    from contextlib import ExitStack

    import concourse.bacc as bacc
    import concourse.bass as bass
    import concourse.tile as tile
    from concourse import bass_isa, mybir

    f32 = mybir.dt.float32
    f16 = mybir.dt.float16
    i32 = mybir.dt.int32
    AF = mybir.ActivationFunctionType
    OP = mybir.AluOpType
    ts = bass.ts

    nc = bacc.Bacc("TRN2", target_bir_lowering=False, debug=False)

    qT = nc.dram_tensor("qT", [HPC, 128, S], f16, kind="ExternalInput").ap()
    kT = nc.dram_tensor("kT", [HPC, 128, S], f16, kind="ExternalInput").ap()
    vp = nc.dram_tensor("vp", [HPC, 128, KB * 65], f16, kind="ExternalInput").ap()
    # per-head (gamma', beta', lam) columns; row 0 of the lam column is 1.0
    gb = nc.dram_tensor("gb", [HPC, 64, 3], f32, kind="ExternalInput").ap()
    outT = nc.dram_tensor("outT", [HPC, 64, S], f32, kind="ExternalOutput").ap()

    with tile.TileContext(nc) as tc, ExitStack() as ctx:
        pq = ctx.enter_context(tc.tile_pool(name="pq", bufs=2))
        pk = ctx.enter_context(tc.tile_pool(name="pk", bufs=2))
        pv = ctx.enter_context(tc.tile_pool(name="pv", bufs=2))
        pe = ctx.enter_context(tc.tile_pool(name="pe", bufs=4))
        psa = ctx.enter_context(tc.tile_pool(name="psa", bufs=2))
        pep = ctx.enter_context(tc.tile_pool(name="pep", bufs=2))
        pout = ctx.enter_context(tc.tile_pool(name="pout", bufs=2))
        pst = ctx.enter_context(tc.tile_pool(name="pst", bufs=2))
        psingle = ctx.enter_context(tc.tile_pool(name="psingle", bufs=1))
        psc = ctx.enter_context(tc.tile_pool(name="psc", bufs=2, space="PSUM"))
        pacc = ctx.enter_context(tc.tile_pool(name="pacc", bufs=1, space="PSUM"))

        eps_t = psingle.tile([65, 1], f32)
        nc.vector.memset(eps_t, EPS)
        ones65 = psingle.tile([1, 65], f32)
        nc.vector.memset(ones65, 1.0)
        junk = psingle.tile([65, 1], f32)
        nc.vector.memset(junk, 1.0)
        magic = psingle.tile([65, 1], i32)
        nc.vector.memset(magic, 0x5F3759DF)

        # PE warm-up: ~24 tiny back-to-back matmuls flip the HAM clock
        # gate toward 8/8 while the first head's DMAs are in flight. The
        # warm-up accumulator borrows the a1 slot; the first chunk's a1
        # allocation simply waits for the last warm-up matmul.
        wu_w = psingle.tile([128, 128], f16)
        nc.vector.memset(wu_w, 0.0)
        wu_ps = pacc.tile([128, 128], f32, tag="a1")
        for _ in range(N_WARMUP_MM):
            nc.tensor.matmul(
                wu_ps[:], lhsT=wu_w[:], rhs=wu_w[:], start=True, stop=True
            )

        def emit_loads(h, startup=False):
            """DMA in head h's tensors. Order front-loads what the first
            key blocks need: K[0:256] (kb0-1), Q chunk 0, V' piece 0. At
            startup the loads spread over three queues (all engines idle)
            so the transfers run in parallel; the mid-stream prefetch for
            the next head stays on the Sync queue (it is fully hidden)."""
            ksh = [pk.tile([128, S // 2], f16, tag=f"ks{j}", name="ks") for j in range(2)]
            qsh = [pq.tile([128, QC], f16, tag=f"qs{j}", name="qs") for j in range(N_QC)]
            vs = pv.tile([128, KB * 65], f16, tag="v")
            vw = KB * 65 // 4
            vq = nc.sync
            nc.sync.dma_start(ksh[0][:, 0:256], kT[h, :, 0:256])
            nc.sync.dma_start(qsh[0][:], qT[h, :, 0:QC])
            vq.dma_start(vs[:, 0:vw], vp[h, :, 0:vw])
            nc.sync.dma_start(ksh[0][:, 256 : S // 2], kT[h, :, 256 : S // 2])
            vq.dma_start(vs[:, vw : 2 * vw], vp[h, :, vw : 2 * vw])
            nc.sync.dma_start(ksh[1][:], kT[h, :, S // 2 : S])
            nc.sync.dma_start(vs[:, 2 * vw : 3 * vw], vp[h, :, 2 * vw : 3 * vw])
            nc.sync.dma_start(vs[:, 3 * vw :], vp[h, :, 3 * vw :])
            for j in range(1, N_QC):
                nc.sync.dma_start(qsh[j][:], qT[h, :, j * QC : (j + 1) * QC])
            gbs = pst.tile([65, 3], f32, tag="gbs")
            nc.vector.memset(gbs[0:1, :], 0.0)
            nc.vector.memset(gbs[0:1, 2:3], 1.0)
            nc.gpsimd.dma_start(gbs[1:65, :], gb[h])
            return ksh, qsh, vs, gbs

        # Deferred per-head tail: the previous head's last-tile PVs,
        # epilogue and finalize are emitted a few QK pairs into the NEXT
        # head's stream, so they don't sit between the last act and the
        # next head's first QKs in the in-order PE queue (same fix as the
        # chunk-boundary PV deferral, applied at the head seam).
        prev_tail = [None]

        def run_head(h, loads):
            ksh, qsh, vs, gbs = loads
            nxt_loads = None

            # [denominator(row 0) | numerator(rows 1-64)] x all queries
            sa1 = psa.tile([65, S], f32)
            sa2 = psa.tile([65, S], f32)
            outc = pout.tile([65, S], f32)
            st = pst.tile([65, N_QC, 6], f32)

            # One flat stream of 512-col score units over the whole head:
            # unit u = qc*32 + 2k + half. Units pack 3 per PSUM tile so
            # each exp covers 1536 cols; tiles may span chunk boundaries
            # (exp doesn't care). PVs trail their act by one tile, and a
            # chunk's first PVs are deferred one act further so the
            # previous chunk's accumulator eviction can drain first.
            UH = N_QC * 2 * KB  # 128 units
            NTH = (UH + 2) // 3  # 43 tiles
            scs = {}
            acc = [None] * N_QC
            pend = []  # units whose act is emitted but PV is not
            n_acts = 0
            last_h = h == HPC - 1

            def u_decode(u):
                qc, r = divmod(u, 2 * KB)
                k, half = divmod(r, 2)
                return qc, k, half

            def emit_epilogue(qc):
                a1, a2 = acc[qc]
                last = last_h and qc == N_QC - 1
                # evict accumulators to SBUF; the next chunk's first PVs
                # are deferred one extra act so this drain can finish.
                nc.vector.tensor_copy(sa1[:, ts(qc, QC)], a1[:])
                if last:
                    nc.scalar.copy(sa2[:, ts(qc, QC)], a2[:])
                    # pre-load the Sqrt act table while the epilogue runs.
                    # The dummy must DEPEND on tail data (the sa2 eviction):
                    # with only an early dep it bypasses the queued exp acts
                    # via the wait-queue and runs at startup, so the real
                    # Sqrt pays a fresh table load on the critical chain.
                    nc.scalar.activation(
                        junk[:], sa2[:, qc * QC : qc * QC + 1], AF.Sqrt
                    )
                else:
                    nc.vector.tensor_copy(sa2[:, ts(qc, QC)], a2[:])

                rb1 = pep.tile([65, QC], f32)
                nc.gpsimd.partition_broadcast(
                    rb1[:], sa1[0:1, ts(qc, QC)], channels=65
                )
                rb2 = pep.tile([65, QC], f32)
                if last:
                    rb2_ps = pacc.tile([65, QC], f32, tag="a1")
                    nc.tensor.matmul(
                        rb2_ps[:],
                        lhsT=ones65[:],
                        rhs=sa2[0:1, ts(qc, QC)],
                        start=True,
                        stop=True,
                    )
                    nc.vector.reciprocal_approx_fast(rb2[:], rb2_ps[:])
                else:
                    nc.gpsimd.partition_broadcast(
                        rb2[:], sa2[0:1, ts(qc, QC)], channels=65
                    )
                    nc.vector.reciprocal_approx_fast(rb2[:], rb2[:])
                nc.vector.reciprocal_approx_fast(rb1[:], rb1[:])
                t1 = pep.tile([65, QC], f32)
                nc.vector.tensor_mul(t1[:], sa1[:, ts(qc, QC)], rb1[:])
                t2 = pep.tile([65, QC], f32)
                nc.vector.tensor_mul(t2[:], sa2[:, ts(qc, QC)], rb2[:])
                # outc = t1 - lam * t2  (row 0: lam-col is 1.0 -> exact 0)
                nc.vector.ln_bwd_dx(
                    outc[:, ts(qc, QC)],
                    dy=t1[:],
                    x_hat=t2[:],
                    mean_dyx=gbs[:, 2:3],
                    mean_dy=0.0,
                    scale=1.0,
                )
                nc.vector.bn_stats(st[:, qc, :], outc[:, ts(qc, QC)])

            def flush_pvs():
                # Emit PVs for pending units. A chunk's first units are
                # held until one act beyond the tile that contains them.
                while pend:
                    u = pend[0]
                    qc, k, half = u_decode(u)
                    # normal lag: one act beyond the unit's own tile;
                    # chunk-first units: one further, so the previous
                    # chunk's accumulator eviction can drain.
                    req = u // 3 + 2 + (1 if k == 0 else 0)
                    if n_acts < req:
                        break
                    if acc[qc] is None:
                        acc[qc] = (
                            pacc.tile([65, QC], f32, tag="a1", name="a1"),
                            pacc.tile([65, QC], f32, tag="a2", name="a2"),
                        )
                    t, i = divmod(u, 3)
                    e = scs[t][1]
                    nc.tensor.matmul(
                        acc[qc][half][:],
                        lhsT=vs[:, ts(k, 65)],
                        rhs=e[:, i * QC : (i + 1) * QC],
                        start=(k == 0),
                        stop=(k == KB - 1),
                    )
                    pend.pop(0)
                    if k == KB - 1 and half == 1:
                        emit_epilogue(qc)
                        acc[qc] = None

            next_act = 0
            for u in range(UH):
                qc, k, half = u_decode(u)
                t, off = divmod(u, 3)
                if off == 0:
                    scs[t] = (
                        psc.tile([128, 3 * QC], f32, tag="sc", name="sc_t"),
                        pe.tile([128, 3 * QC], f16, name="e_t"),
                    )
                ksk = ksh[k // 8][:, ts(k % 8, 128)]
                nc.tensor.matmul(
                    scs[t][0][:, off * QC : (off + 1) * QC],
                    lhsT=ksk[64 * half : 64 * half + 64, :],
                    rhs=qsh[qc][64 * half : 64 * half + 64, :],
                    start=True,
                    stop=True,
                )
                # after three full QK pairs (acts t0/t1 covered), emit the
                # previous head's deferred tail
                if half == 1 and u == 5 and prev_tail[0] is not None:
                    prev_tail[0]()
                    prev_tail[0] = None
                # prefetch the next head's tensors mid-stream, clear of
                # both this head's loads and its finalize out-DMAs; only
                # between QK pairs so the pair stays PE-adjacent
                if half == 1 and u == UH // 2 + 1 and h + 1 < HPC:
                    nxt_loads = emit_loads(h + 1)
                if half == 1:
                    while next_act < NTH and min(3 * next_act + 2, UH - 1) <= u:
                        ta = next_act
                        t_last = min(3 * ta + 2, UH - 1)
                        sc, e = scs[ta]
                        w = (t_last - 3 * ta + 1) * QC
                        nc.scalar.activation(
                            e[:, 0:w], sc[:, 0:w], AF.Exp, scale=SCALE
                        )
                        n_acts += 1
                        pend.extend(range(3 * ta, t_last + 1))
                        next_act += 1
                        flush_pvs()
            flush_pvs()

            def drain_and_finalize():
                while pend:
                    u = pend.pop(0)
                    qc, k, half = u_decode(u)
                    if acc[qc] is None:
                        acc[qc] = (
                            pacc.tile([65, QC], f32, tag="a1", name="a1"),
                            pacc.tile([65, QC], f32, tag="a2", name="a2"),
                        )
                    t, i = divmod(u, 3)
                    e = scs[t][1]
                    nc.tensor.matmul(
                        acc[qc][half][:],
                        lhsT=vs[:, ts(k, 65)],
                        rhs=e[:, i * QC : (i + 1) * QC],
                        start=(k == 0),
                        stop=(k == KB - 1),
                    )
                    if k == KB - 1 and half == 1:
                        emit_epilogue(qc)
                        acc[qc] = None

                # ---- head finalize (partition 0 rows: harmless zeros) ----
                mv = pst.tile([65, 2], f32)
                nc.vector.bn_aggr(mv[:], st[:])
                s2 = pst.tile([65, 2], f32)
                nc.vector.tensor_copy(s2[:, 0:1], mv[:, 0:1])
                # E[x^2]_p = var_p + mean_p^2
                nc.vector.tensor_scalar(
                    out=s2[:, 1:2],
                    in0=mv[:, 0:1],
                    scalar1=mv[:, 0:1],
                    scalar2=mv[:, 1:2],
                    op0=OP.mult,
                    op1=OP.add,
                )
                tot = pst.tile([65, 2], f32)
                nc.gpsimd.partition_all_reduce(
                    tot[:], s2[:], channels=65, reduce_op=bass_isa.ReduceOp.add
                )
                # tot = sums over partitions of per-partition (mean, E[x^2])
                # over 2048 elements; rows 1-64 carry signal -> /64.
                mu = pst.tile([65, 1], f32)
                nc.vector.tensor_scalar_mul(mu[:], tot[:, 0:1], 1.0 / 64.0)
                # veps = (tot1 - tot0*mu - (-64*eps))/64 = var + eps, fused
                # into one ln_bwd_dx pass (tot0*mu = 64*mu^2).
                veps = pst.tile([65, 1], f32)
                nc.vector.ln_bwd_dx(
                    veps[:],
                    dy=tot[:, 1:2],
                    x_hat=tot[:, 0:1],
                    mean_dyx=mu[:],
                    mean_dy=-64.0 * EPS,
                    scale=1.0 / 64.0,
                )
                if last_h:
                    # rstd = sqrt(1/veps): fast DVE reciprocal + ScalarE
                    # sqrt (table pre-loaded right after the last exp act;
                    # ScalarE is idle in the tail).
                    rv = pst.tile([65, 1], f32)
                    nc.vector.reciprocal_approx_fast(rv[:], veps[:])
                    rstd = pst.tile([65, 1], f32)
                    nc.scalar.activation(rstd[:], rv[:], AF.Sqrt)
                    cur = rstd[:]
                else:
                    # rstd = Quake-rsqrt on DVE (bitcast + Newton) so
                    # ScalarE stays on the exp table mid-stream.
                    ish = pst.tile([65, 1], i32)
                    nc.vector.tensor_scalar(
                        out=ish[:],
                        in0=veps[:].bitcast(i32),
                        scalar1=1,
                        scalar2=None,
                        op0=OP.logical_shift_right,
                    )
                    iy = pst.tile([65, 1], i32)
                    nc.vector.tensor_sub(iy[:], magic[:], ish[:])
                    vh = pst.tile([65, 1], f32)
                    nc.vector.tensor_scalar_mul(vh[:], veps[:], -0.5)
                    cur = iy[:].bitcast(f32)
                    for it in range(2):
                        aa = pst.tile([65, 1], f32, tag=f"nr_a{it}")
                        nc.vector.tensor_mul(aa[:], cur, cur)
                        bb = pst.tile([65, 1], f32, tag=f"nr_b{it}")
                        nc.vector.tensor_scalar(
                            out=bb[:], in0=aa[:], scalar1=vh[:], scalar2=1.5,
                            op0=OP.mult, op1=OP.add,
                        )
                        nxt = pst.tile([65, 1], f32, tag=f"nr_y{it}")
                        nc.vector.tensor_tensor(
                            out=nxt[:], in0=bb[:], in1=cur, op=OP.mult
                        )
                        cur = nxt[:]
                sg = pst.tile([65, 1], f32)
                nc.vector.tensor_tensor(
                    out=sg[:], in0=cur, in1=gbs[:, 0:1], op=OP.mult
                )
                tb = pst.tile([65, 1], f32)
                ms = pst.tile([65, 1], f32)
                nc.vector.tensor_scalar(
                    out=ms[:], in0=mu[:], scalar1=sg[:], scalar2=None, op0=OP.mult
                )
                nc.vector.tensor_sub(tb[:], gbs[:, 1:2], ms[:])
                # final affine, in pieces so each piece's output DMA
                # overlaps the next piece's apply. In the exposed tail
                # (last head) pieces alternate ScalarE/DVE and the DMAs
                # alternate queues; mid-stream heads stay off ScalarE.
                n_pieces = 4 if last_h else 2
                outf = pout.tile([65, S], f32)
                for piece in range(n_pieces):
                    sl = slice(
                        piece * (S // n_pieces), (piece + 1) * (S // n_pieces)
                    )
                    if last_h and piece % 2 == 0:
                        nc.scalar.activation(
                            outf[:, sl], outc[:, sl], AF.Identity,
                            bias=tb[:], scale=sg[:],
                        )
                    else:
                        nc.vector.tensor_scalar(
                            out=outf[:, sl],
                            in0=outc[:, sl],
                            scalar1=sg[:],
                            scalar2=tb[:],
                            op0=OP.mult,
                            op1=OP.add,
                        )
                    if last_h and piece % 2 == 1:
                        nc.gpsimd.dma_start(outT[h, :, sl], outf[1:65, sl])
                    else:
                        nc.sync.dma_start(outT[h, :, sl], outf[1:65, sl])

            prev_tail[0] = drain_and_finalize
            return nxt_loads

        lds = emit_loads(0, startup=True)
        for h in range(HPC):
            lds = run_head(h, lds)
        prev_tail[0]()

    nc.compile()
    return nc


def _get_nc():
    if "nc" not in _CACHE:
        _CACHE["nc"] = _build_nc()
    return _CACHE["nc"]


def _host_prep(q, k, v, lq1, lq2, lk1, lk2, gamma, beta):
    """Build per-core input maps."""
    q = np.asarray(q, dtype=np.float32)
    k = np.asarray(k, dtype=np.float32)
    v = np.asarray(v, dtype=np.float32)
    lam = float(
        np.exp(np.float32(np.dot(lq1, lk1)))
        - np.exp(np.float32(np.dot(lq2, lk2)))
        + LAMBDA_INIT
    )
    g2 = (np.asarray(gamma, np.float32) * (1.0 - LAMBDA_INIT)).reshape(H, D)
    b2 = (np.asarray(beta, np.float32) * (1.0 - LAMBDA_INIT)).reshape(H, D)

    in_maps = []
    for c in range(N_CORES):
        heads = range(c * HPC, (c + 1) * HPC)
        qTa = np.empty((HPC, 128, S), np.float16)
        kTa = np.empty((HPC, 128, S), np.float16)
        vpa = np.empty((HPC, 128, KB * 65), np.float16)
        gba = np.empty((HPC, 64, 3), np.float32)
        for i, hh in enumerate(heads):
            qTa[i] = q[0, hh].T.astype(np.float16)
            kTa[i] = k[0, hh].T.astype(np.float16)
            vh = v[0, hh]  # [S, 64]
            v1 = np.concatenate([np.ones((S, 1), np.float32), vh], axis=1)
            # SBUF image: [partition(key within block), kblock*65 + col]
            vpa[i] = (
                v1.reshape(KB, 128, 65).transpose(1, 0, 2).reshape(128, KB * 65)
            ).astype(np.float16)
            gba[i, :, 0] = g2[hh]
            gba[i, :, 1] = b2[hh]
            gba[i, :, 2] = lam
        in_maps.append({"qT": qTa, "kT": kTa, "vp": vpa, "gb": gba})
    return in_maps


def kernel(q, k, v, lq1, lq2, lk1, lk2, gamma, beta, _trace=False, _tmpdir=None):
    from concourse.bass_utils import run_bass_kernel_spmd

    nc = _get_nc()
    in_maps = _host_prep(q, k, v, lq1, lq2, lk1, lk2, gamma, beta)
    res = run_bass_kernel_spmd(
        nc,
        in_maps,
        core_ids=list(range(N_CORES)),
        trace=_trace,
        tmpdir=_tmpdir,
    )
    out = np.empty((B, H, S, D), np.float32)
    for c in range(N_CORES):
        outT = res.results[c]["outT"]  # [HPC, 64, S]
        for i in range(HPC):
            out[0, c * HPC + i] = outT[i].T
    if _trace:
        _CACHE["last_results"] = res
    return out



# revision 5
# speedup vs baseline: 1.0832x; 1.0832x over previous
"""Differential attention (two-softmax diff + GroupNorm) on 8 TRN2 cores.

Sharding: 16 heads / 8 cores = 2 heads per core (head-parallel, no
collectives). GroupNorm stats are per-(batch, head) so each core is fully
independent.

Device layout choices (host prepares everything):
  - Q, K per head are host-transposed to [128(d), 2048(s)] fp16: partitions
    0-63 hold half-1 (q1/k1), partitions 64-127 hold half-2. QK^T then
    contracts over the partition dim directly, producing transposed score
    blocks S^T[key, query] in PSUM (fp32). The two 64-contraction QK
    matmuls issue back-to-back and the PE runs them concurrently on row
    groups h0/h64.
  - V per head is prefixed with a ones column (V' = [1 | V], 65 cols, fp16)
    and pre-arranged into the SBUF image [128(key of block), 16*65]: the PV
    matmul (lhsT = V'[kblk], rhs = exp(S^T)[kblk]) then yields the softmax
    denominator on partition 0 and the numerator on partitions 1-64 in one
    accumulation group. BOTH halves share the same V' stationary (w1*V and
    lam*w2*V use the same V); lam is applied in the per-chunk epilogue via
    the fused LN_BWD_DX DVE op (out = t1 - lam*t2).
  - Output stays in [d, q] layout on device; the host transposes it back.

fp16 is used on every matmul path: it streams through the PE at ~1
cycle/column with the same 10-bit mantissa class as tf32. exp() runs on
ScalarE straight out of PSUM, writing fp16.

Main loop per (head, 512-query chunk): 16 key blocks of
  QK matmul pair -> exp on ScalarE (PSUM -> SBUF) -> PV pair accumulate,
then a per-chunk epilogue slice (denominator broadcast on GpSimd, divide +
combine + bn_stats on DVE) that hides under later chunks' main loop.
ScalarE's exp stream is the pacing engine (~1.1us per key block); the PE
fits just beneath it, so no warm-up spinner is used (the HAM clock gate
settles by itself and junk matmuls only delay the first real block).

Tail: rstd = sqrt(reciprocal_approx_fast(var+eps)) with the Sqrt act
table pre-loaded via a dummy activation right after the last exp, and the
final affine + output DMA run in 4 interleaved pieces.
"""

import math

import numpy as np

B, H, S, D = 1, 16, 2048, 64
N_CORES = 8
HPC = H // N_CORES  # heads per core
QC = 512            # query-chunk width (PSUM bank budget)
N_QC = S // QC
KB = S // 128       # key blocks of 128
LAMBDA_INIT = 0.8
EPS = 1e-5
SCALE = 1.0 / math.sqrt(D)
N_WARMUP_MM = 10

_CACHE = {}


def _build_nc():
    from contextlib import ExitStack

    import concourse.bacc as bacc
    import concourse.bass as bass
    import concourse.tile as tile
    from concourse import bass_isa, mybir

    f32 = mybir.dt.float32
    f16 = mybir.dt.float16
    i32 = mybir.dt.int32
    AF = mybir.ActivationFunctionType
    OP = mybir.AluOpType
    ts = bass.ts

    nc = bacc.Bacc("TRN2", target_bir_lowering=False, debug=False)

    qT = nc.dram_tensor("qT", [HPC, 128, S], f16, kind="ExternalInput").ap()
    kT = nc.dram_tensor("kT", [HPC, 128, S], f16, kind="ExternalInput").ap()
    vp = nc.dram_tensor("vp", [HPC, 128, KB * 65], f16, kind="ExternalInput").ap()
    # per-head (gamma', beta', lam) columns; row 0 of the lam column is 1.0
    gb = nc.dram_tensor("gb", [HPC, 64, 3], f32, kind="ExternalInput").ap()
    outT = nc.dram_tensor("outT", [HPC, 64, S], f32, kind="ExternalOutput").ap()

    with tile.TileContext(nc) as tc, ExitStack() as ctx:
        pq = ctx.enter_context(tc.tile_pool(name="pq", bufs=2))
        pk = ctx.enter_context(tc.tile_pool(name="pk", bufs=2))
        pv = ctx.enter_context(tc.tile_pool(name="pv", bufs=2))
        pe = ctx.enter_context(tc.tile_pool(name="pe", bufs=4))
        psa = ctx.enter_context(tc.tile_pool(name="psa", bufs=2))
        pep = ctx.enter_context(tc.tile_pool(name="pep", bufs=2))
        pout = ctx.enter_context(tc.tile_pool(name="pout", bufs=2))
        pst = ctx.enter_context(tc.tile_pool(name="pst", bufs=2))
        psingle = ctx.enter_context(tc.tile_pool(name="psingle", bufs=1))
        psc = ctx.enter_context(tc.tile_pool(name="psc", bufs=2, space="PSUM"))
        pacc = ctx.enter_context(tc.tile_pool(name="pacc", bufs=1, space="PSUM"))

        eps_t = psingle.tile([65, 1], f32)
        nc.vector.memset(eps_t, EPS)
        ones65 = psingle.tile([1, 65], f32)
        nc.vector.memset(ones65, 1.0)
        junk = psingle.tile([65, 1], f32)
        nc.vector.memset(junk, 1.0)
        magic = psingle.tile([65, 1], i32)
        nc.vector.memset(magic, 0x5F3759DF)

        # PE warm-up: ~24 tiny back-to-back matmuls flip the HAM clock
        # gate toward 8/8 while the first head's DMAs are in flight. The
        # warm-up accumulator borrows the a1 slot; the first chunk's a1
        # allocation simply waits for the last warm-up matmul.
        wu_w = psingle.tile([128, 128], f16)
        nc.vector.memset(wu_w, 0.0)
        wu_ps = pacc.tile([128, 128], f32, tag="a1")
        for _ in range(N_WARMUP_MM):
            nc.tensor.matmul(
                wu_ps[:], lhsT=wu_w[:], rhs=wu_w[:], start=True, stop=True
            )

        def emit_loads(h, startup=False):
            """DMA in head h's tensors. At startup the loads spread over
            three queues (sync: K, gpsimd: Q + gb, vector: V') so the
            transfers run in parallel and the first key blocks' data
            (K[0:256], Q chunk 0) leads each queue; the mid-stream
            prefetch for the next head stays on the Sync queue (it is
            fully hidden under the exp stream)."""
            ksh = [pk.tile([128, S // 2], f16, tag=f"ks{j}", name="ks") for j in range(2)]
            qsh = [pq.tile([128, QC], f16, tag=f"qs{j}", name="qs") for j in range(N_QC)]
            vs = pv.tile([128, KB * 65], f16, tag="v")
            vw = KB * 65 // 4
            kq = nc.sync
            qq = nc.gpsimd if startup else nc.sync
            va = nc.sync
            vb = nc.gpsimd if startup else nc.sync
            kq.dma_start(ksh[0][:, 0:256], kT[h, :, 0:256])
            qq.dma_start(qsh[0][:], qT[h, :, 0:QC])
            kq.dma_start(ksh[0][:, 256 : S // 2], kT[h, :, 256 : S // 2])
            qq.dma_start(qsh[1][:], qT[h, :, QC : 2 * QC])
            va.dma_start(vs[:, 0:vw], vp[h, :, 0:vw])
            vb.dma_start(vs[:, vw : 2 * vw], vp[h, :, vw : 2 * vw])
            kq.dma_start(ksh[1][:], kT[h, :, S // 2 : S])
            va.dma_start(vs[:, 2 * vw : 3 * vw], vp[h, :, 2 * vw : 3 * vw])
            vb.dma_start(vs[:, 3 * vw :], vp[h, :, 3 * vw :])
            for j in range(2, N_QC):
                qq.dma_start(qsh[j][:], qT[h, :, j * QC : (j + 1) * QC])
            gbs = pst.tile([65, 3], f32, tag="gbs")
            nc.vector.memset(gbs[0:1, :], 0.0)
            nc.vector.memset(gbs[0:1, 2:3], 1.0)
            nc.gpsimd.dma_start(gbs[1:65, :], gb[h])
            return ksh, qsh, vs, gbs

        # Deferred per-head tail: the previous head's last-tile PVs,
        # epilogue and finalize are emitted a few QK pairs into the NEXT
        # head's stream, so they don't sit between the last act and the
        # next head's first QKs in the in-order PE queue (same fix as the
        # chunk-boundary PV deferral, applied at the head seam).
        prev_tail = [None]

        def run_head(h, loads):
            ksh, qsh, vs, gbs = loads
            nxt_loads = None

            # [denominator(row 0) | numerator(rows 1-64)] x all queries
            sa1 = psa.tile([65, S], f32)
            sa2 = psa.tile([65, S], f32)
            outc = pout.tile([65, S], f32)
            st = pst.tile([65, N_QC, 6], f32)

            # One flat stream of 512-col score units over the whole head:
            # unit u = qc*32 + 2k + half. Units pack 3 per PSUM tile so
            # each exp covers 1536 cols; tiles may span chunk boundaries
            # (exp doesn't care). PVs trail their act by one tile, and a
            # chunk's first PVs are deferred one act further so the
            # previous chunk's accumulator eviction can drain first.
            UH = N_QC * 2 * KB  # 128 units
            NTH = (UH + 2) // 3  # 43 tiles
            scs = {}
            acc = [None] * N_QC
            pend = []  # units whose act is emitted but PV is not
            n_acts = 0
            last_h = h == HPC - 1

            def u_decode(u):
                qc, r = divmod(u, 2 * KB)
                k, half = divmod(r, 2)
                return qc, k, half

            def emit_epilogue(qc):
                a1, a2 = acc[qc]
                last = last_h and qc == N_QC - 1
                # evict accumulators to SBUF; the next chunk's first PVs
                # are deferred one extra act so this drain can finish.
                nc.vector.tensor_copy(sa1[:, ts(qc, QC)], a1[:])
                if last:
                    nc.scalar.copy(sa2[:, ts(qc, QC)], a2[:])
                    # pre-load the Sqrt act table while the epilogue runs.
                    # The dummy must DEPEND on tail data (the sa2 eviction):
                    # with only an early dep it bypasses the queued exp acts
                    # via the wait-queue and runs at startup, so the real
                    # Sqrt pays a fresh table load on the critical chain.
                    nc.scalar.activation(
                        junk[:], sa2[:, qc * QC : qc * QC + 1], AF.Sqrt
                    )
                else:
                    nc.vector.tensor_copy(sa2[:, ts(qc, QC)], a2[:])

                rb1 = pep.tile([65, QC], f32)
                nc.gpsimd.partition_broadcast(
                    rb1[:], sa1[0:1, ts(qc, QC)], channels=65
                )
                rb2 = pep.tile([65, QC], f32)
                if last:
                    rb2_ps = pacc.tile([65, QC], f32, tag="a1")
                    nc.tensor.matmul(
                        rb2_ps[:],
                        lhsT=ones65[:],
                        rhs=sa2[0:1, ts(qc, QC)],
                        start=True,
                        stop=True,
                    )
                    nc.vector.reciprocal_approx_fast(rb2[:], rb2_ps[:])
                else:
                    nc.gpsimd.partition_broadcast(
                        rb2[:], sa2[0:1, ts(qc, QC)], channels=65
                    )
                    nc.vector.reciprocal_approx_fast(rb2[:], rb2[:])
                nc.vector.reciprocal_approx_fast(rb1[:], rb1[:])
                t1 = pep.tile([65, QC], f32)
                nc.vector.tensor_mul(t1[:], sa1[:, ts(qc, QC)], rb1[:])
                t2 = pep.tile([65, QC], f32)
                nc.vector.tensor_mul(t2[:], sa2[:, ts(qc, QC)], rb2[:])
                # outc = t1 - lam * t2  (row 0: lam-col is 1.0 -> exact 0)
                nc.vector.ln_bwd_dx(
                    outc[:, ts(qc, QC)],
                    dy=t1[:],
                    x_hat=t2[:],
                    mean_dyx=gbs[:, 2:3],
                    mean_dy=0.0,
                    scale=1.0,
                )
                nc.vector.bn_stats(st[:, qc, :], outc[:, ts(qc, QC)])

            def flush_pvs():
                # Emit PVs for pending units. A chunk's first units are
                # held until one act beyond the tile that contains them.
                while pend:
                    u = pend[0]
                    qc, k, half = u_decode(u)
                    # normal lag: two acts beyond the unit's own tile, so
                    # in the in-order PE queue the NEXT tile's QKs precede
                    # these PVs (which block on the previous act's exp +
                    # 100ns sem propagation) — the following act's QK
                    # dependency then resolves well before the act engine
                    # is free, instead of ~150ns late. Chunk-first units:
                    # one further, so the previous chunk's accumulator
                    # eviction can drain.
                    req = u // 3 + 3 + (1 if k == 0 else 0)
                    if n_acts < req:
                        break
                    if acc[qc] is None:
                        acc[qc] = (
                            pacc.tile([65, QC], f32, tag="a1", name="a1"),
                            pacc.tile([65, QC], f32, tag="a2", name="a2"),
                        )
                    t, i = divmod(u, 3)
                    e = scs[t][1]
                    nc.tensor.matmul(
                        acc[qc][half][:],
                        lhsT=vs[:, ts(k, 65)],
                        rhs=e[:, i * QC : (i + 1) * QC],
                        start=(k == 0),
                        stop=(k == KB - 1),
                    )
                    pend.pop(0)
                    if k == KB - 1 and half == 1:
                        emit_epilogue(qc)
                        acc[qc] = None

            next_act = 0
            for u in range(UH):
                qc, k, half = u_decode(u)
                t, off = divmod(u, 3)
                if off == 0:
                    scs[t] = (
                        psc.tile([128, 3 * QC], f32, tag="sc", name="sc_t"),
                        pe.tile([128, 3 * QC], f16, name="e_t"),
                    )
                ksk = ksh[k // 8][:, ts(k % 8, 128)]
                nc.tensor.matmul(
                    scs[t][0][:, off * QC : (off + 1) * QC],
                    lhsT=ksk[64 * half : 64 * half + 64, :],
                    rhs=qsh[qc][64 * half : 64 * half + 64, :],
                    start=True,
                    stop=True,
                )
                # after three full QK pairs (acts t0/t1 covered), emit the
                # previous head's deferred tail
                if half == 1 and u == 5 and prev_tail[0] is not None:
                    prev_tail[0]()
                    prev_tail[0] = None
                # prefetch the next head's tensors mid-stream, clear of
                # both this head's loads and its finalize out-DMAs; only
                # between QK pairs so the pair stays PE-adjacent
                if half == 1 and u == UH // 2 + 1 and h + 1 < HPC:
                    nxt_loads = emit_loads(h + 1)
                if half == 1:
                    while next_act < NTH and min(3 * next_act + 2, UH - 1) <= u:
                        ta = next_act
                        t_last = min(3 * ta + 2, UH - 1)
                        sc, e = scs[ta]
                        w = (t_last - 3 * ta + 1) * QC
                        nc.scalar.activation(
                            e[:, 0:w], sc[:, 0:w], AF.Exp, scale=SCALE
                        )
                        n_acts += 1
                        pend.extend(range(3 * ta, t_last + 1))
                        next_act += 1
                        flush_pvs()
            flush_pvs()

            def drain_and_finalize():
                while pend:
                    u = pend.pop(0)
                    qc, k, half = u_decode(u)
                    if acc[qc] is None:
                        acc[qc] = (
                            pacc.tile([65, QC], f32, tag="a1", name="a1"),
                            pacc.tile([65, QC], f32, tag="a2", name="a2"),
                        )
                    t, i = divmod(u, 3)
                    e = scs[t][1]
                    nc.tensor.matmul(
                        acc[qc][half][:],
                        lhsT=vs[:, ts(k, 65)],
                        rhs=e[:, i * QC : (i + 1) * QC],
                        start=(k == 0),
                        stop=(k == KB - 1),
                    )
                    if k == KB - 1 and half == 1:
                        emit_epilogue(qc)
                        acc[qc] = None

                # ---- head finalize (partition 0 rows: harmless zeros) ----
                mv = pst.tile([65, 2], f32)
                nc.vector.bn_aggr(mv[:], st[:])
                s2 = pst.tile([65, 2], f32)
                nc.vector.tensor_copy(s2[:, 0:1], mv[:, 0:1])
                # E[x^2]_p = var_p + mean_p^2
                nc.vector.tensor_scalar(
                    out=s2[:, 1:2],
                    in0=mv[:, 0:1],
                    scalar1=mv[:, 0:1],
                    scalar2=mv[:, 1:2],
                    op0=OP.mult,
                    op1=OP.add,
                )
                tot = pst.tile([65, 2], f32)
                nc.gpsimd.partition_all_reduce(
                    tot[:], s2[:], channels=65, reduce_op=bass_isa.ReduceOp.add
                )
                # tot = sums over partitions of per-partition (mean, E[x^2])
                # over 2048 elements; rows 1-64 carry signal -> /64.
                mu = pst.tile([65, 1], f32)
                nc.vector.tensor_scalar_mul(mu[:], tot[:, 0:1], 1.0 / 64.0)
                # veps = (tot1 - tot0*mu - (-64*eps))/64 = var + eps, fused
                # into one ln_bwd_dx pass (tot0*mu = 64*mu^2).
                veps = pst.tile([65, 1], f32)
                nc.vector.ln_bwd_dx(
                    veps[:],
                    dy=tot[:, 1:2],
                    x_hat=tot[:, 0:1],
                    mean_dyx=mu[:],
                    mean_dy=-64.0 * EPS,
                    scale=1.0 / 64.0,
                )
                if last_h:
                    # rstd = sqrt(1/veps): fast DVE reciprocal + ScalarE
                    # sqrt (table pre-loaded right after the last exp act;
                    # ScalarE is idle in the tail).
                    rv = pst.tile([65, 1], f32)
                    nc.vector.reciprocal_approx_fast(rv[:], veps[:])
                    rstd = pst.tile([65, 1], f32)
                    nc.scalar.activation(rstd[:], rv[:], AF.Sqrt)
                    cur = rstd[:]
                else:
                    # rstd = Quake-rsqrt on DVE (bitcast + Newton) so
                    # ScalarE stays on the exp table mid-stream.
                    ish = pst.tile([65, 1], i32)
                    nc.vector.tensor_scalar(
                        out=ish[:],
                        in0=veps[:].bitcast(i32),
                        scalar1=1,
                        scalar2=None,
                        op0=OP.logical_shift_right,
                    )
                    iy = pst.tile([65, 1], i32)
                    nc.vector.tensor_sub(iy[:], magic[:], ish[:])
                    vh = pst.tile([65, 1], f32)
                    nc.vector.tensor_scalar_mul(vh[:], veps[:], -0.5)
                    cur = iy[:].bitcast(f32)
                    for it in range(2):
                        aa = pst.tile([65, 1], f32, tag=f"nr_a{it}")
                        nc.vector.tensor_mul(aa[:], cur, cur)
                        bb = pst.tile([65, 1], f32, tag=f"nr_b{it}")
                        nc.vector.tensor_scalar(
                            out=bb[:], in0=aa[:], scalar1=vh[:], scalar2=1.5,
                            op0=OP.mult, op1=OP.add,
                        )
                        nxt = pst.tile([65, 1], f32, tag=f"nr_y{it}")
                        nc.vector.tensor_tensor(
                            out=nxt[:], in0=bb[:], in1=cur, op=OP.mult
                        )
                        cur = nxt[:]
                sg = pst.tile([65, 1], f32)
                nc.vector.tensor_tensor(
                    out=sg[:], in0=cur, in1=gbs[:, 0:1], op=OP.mult
                )
                tb = pst.tile([65, 1], f32)
                ms = pst.tile([65, 1], f32)
                nc.vector.tensor_scalar(
                    out=ms[:], in0=mu[:], scalar1=sg[:], scalar2=None, op0=OP.mult
                )
                nc.vector.tensor_sub(tb[:], gbs[:, 1:2], ms[:])
                # final affine, in pieces so each piece's output DMA
                # overlaps the next piece's apply. In the exposed tail
                # (last head) pieces alternate ScalarE/DVE and the DMAs
                # alternate queues; mid-stream heads stay off ScalarE.
                n_pieces = 4 if last_h else 2
                outf = pout.tile([65, S], f32)
                for piece in range(n_pieces):
                    sl = slice(
                        piece * (S // n_pieces), (piece + 1) * (S // n_pieces)
                    )
                    if last_h and piece % 2 == 0:
                        nc.scalar.activation(
                            outf[:, sl], outc[:, sl], AF.Identity,
                            bias=tb[:], scale=sg[:],
                        )
                    else:
                        nc.vector.tensor_scalar(
                            out=outf[:, sl],
                            in0=outc[:, sl],
                            scalar1=sg[:],
                            scalar2=tb[:],
                            op0=OP.mult,
                            op1=OP.add,
                        )
                    if last_h and piece % 2 == 1:
                        nc.gpsimd.dma_start(outT[h, :, sl], outf[1:65, sl])
                    else:
                        nc.sync.dma_start(outT[h, :, sl], outf[1:65, sl])

            prev_tail[0] = drain_and_finalize
            return nxt_loads

        lds = emit_loads(0, startup=True)
        for h in range(HPC):
            lds = run_head(h, lds)
        prev_tail[0]()

    nc.compile()
    return nc


def _get_nc():
    if "nc" not in _CACHE:
        _CACHE["nc"] = _build_nc()
    return _CACHE["nc"]


def _host_prep(q, k, v, lq1, lq2, lk1, lk2, gamma, beta):
    """Build per-core input maps."""
    q = np.asarray(q, dtype=np.float32)
    k = np.asarray(k, dtype=np.float32)
    v = np.asarray(v, dtype=np.float32)
    lam = float(
        np.exp(np.float32(np.dot(lq1, lk1)))
        - np.exp(np.float32(np.dot(lq2, lk2)))
        + LAMBDA_INIT
    )
    g2 = (np.asarray(gamma, np.float32) * (1.0 - LAMBDA_INIT)).reshape(H, D)
    b2 = (np.asarray(beta, np.float32) * (1.0 - LAMBDA_INIT)).reshape(H, D)

    in_maps = []
    for c in range(N_CORES):
        heads = range(c * HPC, (c + 1) * HPC)
        qTa = np.empty((HPC, 128, S), np.float16)
        kTa = np.empty((HPC, 128, S), np.float16)
        vpa = np.empty((HPC, 128, KB * 65), np.float16)
        gba = np.empty((HPC, 64, 3), np.float32)
        for i, hh in enumerate(heads):
            qTa[i] = q[0, hh].T.astype(np.float16)
            kTa[i] = k[0, hh].T.astype(np.float16)
            vh = v[0, hh]  # [S, 64]
            v1 = np.concatenate([np.ones((S, 1), np.float32), vh], axis=1)
            # SBUF image: [partition(key within block), kblock*65 + col]
            vpa[i] = (
                v1.reshape(KB, 128, 65).transpose(1, 0, 2).reshape(128, KB * 65)
            ).astype(np.float16)
            gba[i, :, 0] = g2[hh]
            gba[i, :, 1] = b2[hh]
            gba[i, :, 2] = lam
        in_maps.append({"qT": qTa, "kT": kTa, "vp": vpa, "gb": gba})
    return in_maps


def kernel(q, k, v, lq1, lq2, lk1, lk2, gamma, beta, _trace=False, _tmpdir=None):
    from concourse.bass_utils import run_bass_kernel_spmd

    nc = _get_nc()
    in_maps = _host_prep(q, k, v, lq1, lq2, lk1, lk2, gamma, beta)
    res = run_bass_kernel_spmd(
        nc,
        in_maps,
        core_ids=list(range(N_CORES)),
        trace=_trace,
        tmpdir=_tmpdir,
    )
    out = np.empty((B, H, S, D), np.float32)
    for c in range(N_CORES):
        outT = res.results[c]["outT"]  # [HPC, 64, S]
        for i in range(HPC):
            out[0, c * HPC + i] = outT[i].T
    if _trace:
        _CACHE["last_results"] = res
    return out



# revision 7
# speedup vs baseline: 1.0899x; 1.0062x over previous
"""Differential attention (two-softmax diff + GroupNorm) on 8 TRN2 cores.

Sharding: 16 heads / 8 cores = 2 heads per core (head-parallel, no
collectives). GroupNorm stats are per-(batch, head) so each core is fully
independent.

Device layout choices (host prepares everything):
  - Q, K per head are host-transposed to [128(d), 2048(s)] fp16: partitions
    0-63 hold half-1 (q1/k1), partitions 64-127 hold half-2. QK^T then
    contracts over the partition dim directly, producing transposed score
    blocks S^T[key, query] in PSUM (fp32). The two 64-contraction QK
    matmuls issue back-to-back and the PE runs them concurrently on row
    groups h0/h64.
  - V per head is prefixed with a ones column (V' = [1 | V], 65 cols, fp16)
    and pre-arranged into the SBUF image [128(key of block), 16*65]: the PV
    matmul (lhsT = V'[kblk], rhs = exp(S^T)[kblk]) then yields the softmax
    denominator on partition 0 and the numerator on partitions 1-64 in one
    accumulation group. BOTH halves share the same V' stationary (w1*V and
    lam*w2*V use the same V); lam is applied in the per-chunk epilogue via
    the fused LN_BWD_DX DVE op (out = t1 - lam*t2).
  - Output stays in [d, q] layout on device; the host transposes it back.

fp16 is used on every matmul path: it streams through the PE at ~1
cycle/column with the same 10-bit mantissa class as tf32. exp() runs on
ScalarE straight out of PSUM, writing fp16.

Main loop per (head, 512-query chunk): 16 key blocks of
  QK matmul pair -> exp on ScalarE (PSUM -> SBUF) -> PV pair accumulate,
then a per-chunk epilogue slice (denominator broadcast on GpSimd, divide +
combine + bn_stats on DVE) that hides under later chunks' main loop.
ScalarE's exp stream is the pacing engine (~1.1us per key block); the PE
fits just beneath it, so no warm-up spinner is used (the HAM clock gate
settles by itself and junk matmuls only delay the first real block).

Tail: rstd = sqrt(reciprocal_approx_fast(var+eps)) with the Sqrt act
table pre-loaded via a dummy activation right after the last exp, and the
final affine + output DMA run in 4 interleaved pieces.
"""

import math

import numpy as np

B, H, S, D = 1, 16, 2048, 64
N_CORES = 8
HPC = H // N_CORES  # heads per core
QC = 512            # query-chunk width (PSUM bank budget)
N_QC = S // QC
KB = S // 128       # key blocks of 128
LAMBDA_INIT = 0.8
EPS = 1e-5
SCALE = 1.0 / math.sqrt(D)
N_WARMUP_MM = 10

_CACHE = {}


def _build_nc():
    from contextlib import ExitStack

    import concourse.bacc as bacc
    import concourse.bass as bass
    import concourse.tile as tile
    from concourse import bass_isa, mybir

    f32 = mybir.dt.float32
    f16 = mybir.dt.float16
    i32 = mybir.dt.int32
    AF = mybir.ActivationFunctionType
    OP = mybir.AluOpType
    ts = bass.ts

    nc = bacc.Bacc("TRN2", target_bir_lowering=False, debug=False)

    qT = nc.dram_tensor("qT", [HPC, 128, S], f16, kind="ExternalInput").ap()
    kT = nc.dram_tensor("kT", [HPC, 128, S], f16, kind="ExternalInput").ap()
    vp = nc.dram_tensor("vp", [HPC, 128, KB * 65], f16, kind="ExternalInput").ap()
    # per-head (gamma', beta', lam) columns; row 0 of the lam column is 1.0
    gb = nc.dram_tensor("gb", [HPC, 64, 3], f32, kind="ExternalInput").ap()
    outT = nc.dram_tensor("outT", [HPC, 64, S], f32, kind="ExternalOutput").ap()

    with tile.TileContext(nc) as tc, ExitStack() as ctx:
        pq = ctx.enter_context(tc.tile_pool(name="pq", bufs=2))
        pk = ctx.enter_context(tc.tile_pool(name="pk", bufs=2))
        pv = ctx.enter_context(tc.tile_pool(name="pv", bufs=2))
        pe = ctx.enter_context(tc.tile_pool(name="pe", bufs=4))
        psa = ctx.enter_context(tc.tile_pool(name="psa", bufs=2))
        pep = ctx.enter_context(tc.tile_pool(name="pep", bufs=2))
        pout = ctx.enter_context(tc.tile_pool(name="pout", bufs=2))
        pst = ctx.enter_context(tc.tile_pool(name="pst", bufs=2))
        psingle = ctx.enter_context(tc.tile_pool(name="psingle", bufs=1))
        psc = ctx.enter_context(tc.tile_pool(name="psc", bufs=2, space="PSUM"))
        pacc = ctx.enter_context(tc.tile_pool(name="pacc", bufs=1, space="PSUM"))

        eps_t = psingle.tile([65, 1], f32)
        nc.vector.memset(eps_t, EPS)
        ones65 = psingle.tile([1, 65], f32)
        nc.vector.memset(ones65, 1.0)
        junk = psingle.tile([65, 1], f32)
        nc.vector.memset(junk, 1.0)
        magic = psingle.tile([65, 1], i32)
        nc.vector.memset(magic, 0x5F3759DF)

        # PE warm-up: ~24 tiny back-to-back matmuls flip the HAM clock
        # gate toward 8/8 while the first head's DMAs are in flight. The
        # warm-up accumulator borrows the a1 slot; the first chunk's a1
        # allocation simply waits for the last warm-up matmul.
        wu_w = psingle.tile([128, 128], f16)
        nc.vector.memset(wu_w, 0.0)
        wu_ps = pacc.tile([128, 128], f32, tag="a1")
        for _ in range(N_WARMUP_MM):
            nc.tensor.matmul(
                wu_ps[:], lhsT=wu_w[:], rhs=wu_w[:], start=True, stop=True
            )

        def emit_loads(h, startup=False):
            """DMA in head h's tensors. At startup the loads spread over
            three queues (sync: K, gpsimd: Q + gb, vector: V') so the
            transfers run in parallel and the first key blocks' data
            (K[0:256], Q chunk 0) leads each queue; the mid-stream
            prefetch for the next head stays on the Sync queue (it is
            fully hidden under the exp stream)."""
            ksh = [pk.tile([128, S // 2], f16, tag=f"ks{j}", name="ks") for j in range(2)]
            qsh = [pq.tile([128, QC], f16, tag=f"qs{j}", name="qs") for j in range(N_QC)]
            vs = pv.tile([128, KB * 65], f16, tag="v")
            vw = KB * 65 // 4
            if startup:
                # Per-queue transfer rate is only ~38GB/s (1KB lines), so
                # the plan pipelines each queue in consumption order:
                #   scalar: K[0:256] (the queue is otherwise idle until the
                #           first exp act, which needs this data anyway)
                #   sync:   q0 left half, then K in fine pieces paced just
                #           ahead of the key-block sweep
                #   gpsimd: q0 right half, then V'/Q interleaved
                nc.scalar.dma_start(ksh[0][:, 0:256], kT[h, :, 0:256])
                nc.sync.dma_start(qsh[0][:, 0:256], qT[h, :, 0:256])
                nc.gpsimd.dma_start(qsh[0][:, 256:QC], qT[h, :, 256:QC])
                nc.sync.dma_start(ksh[0][:, 256:512], kT[h, :, 256:512])
                nc.gpsimd.dma_start(vs[:, 0:vw], vp[h, :, 0:vw])
                nc.sync.dma_start(ksh[0][:, 512:768], kT[h, :, 512:768])
                nc.gpsimd.dma_start(qsh[1][:], qT[h, :, QC : 2 * QC])
                nc.sync.dma_start(ksh[0][:, 768:1024], kT[h, :, 768:1024])
                nc.gpsimd.dma_start(vs[:, vw : 2 * vw], vp[h, :, vw : 2 * vw])
                nc.sync.dma_start(ksh[1][:, 0:512], kT[h, :, 1024:1536])
                nc.gpsimd.dma_start(vs[:, 2 * vw : 3 * vw], vp[h, :, 2 * vw : 3 * vw])
                nc.sync.dma_start(ksh[1][:, 512:1024], kT[h, :, 1536:2048])
                nc.gpsimd.dma_start(qsh[2][:], qT[h, :, 2 * QC : 3 * QC])
                nc.gpsimd.dma_start(vs[:, 3 * vw :], vp[h, :, 3 * vw :])
                nc.gpsimd.dma_start(qsh[3][:], qT[h, :, 3 * QC : 4 * QC])
            else:
                nc.sync.dma_start(ksh[0][:, 0:256], kT[h, :, 0:256])
                nc.sync.dma_start(qsh[0][:], qT[h, :, 0:QC])
                nc.sync.dma_start(ksh[0][:, 256 : S // 2], kT[h, :, 256 : S // 2])
                nc.sync.dma_start(qsh[1][:], qT[h, :, QC : 2 * QC])
                nc.sync.dma_start(vs[:, 0:vw], vp[h, :, 0:vw])
                nc.sync.dma_start(vs[:, vw : 2 * vw], vp[h, :, vw : 2 * vw])
                nc.sync.dma_start(ksh[1][:], kT[h, :, S // 2 : S])
                nc.sync.dma_start(vs[:, 2 * vw : 3 * vw], vp[h, :, 2 * vw : 3 * vw])
                nc.sync.dma_start(vs[:, 3 * vw :], vp[h, :, 3 * vw :])
                for j in range(2, N_QC):
                    nc.sync.dma_start(qsh[j][:], qT[h, :, j * QC : (j + 1) * QC])
            gbs = pst.tile([65, 3], f32, tag="gbs")
            nc.vector.memset(gbs[0:1, :], 0.0)
            nc.vector.memset(gbs[0:1, 2:3], 1.0)
            nc.gpsimd.dma_start(gbs[1:65, :], gb[h])
            return ksh, qsh, vs, gbs

        # Deferred per-head tail: the previous head's last-tile PVs,
        # epilogue and finalize are emitted a few QK pairs into the NEXT
        # head's stream, so they don't sit between the last act and the
        # next head's first QKs in the in-order PE queue (same fix as the
        # chunk-boundary PV deferral, applied at the head seam).
        prev_tail = [None]

        def run_head(h, loads):
            ksh, qsh, vs, gbs = loads
            nxt_loads = None

            # [denominator(row 0) | numerator(rows 1-64)] x all queries
            sa1 = psa.tile([65, S], f32)
            sa2 = psa.tile([65, S], f32)
            outc = pout.tile([65, S], f32)
            st = pst.tile([65, N_QC, 6], f32)

            # One flat stream of 512-col score units over the whole head:
            # unit u = qc*32 + 2k + half. Units pack 3 per PSUM tile so
            # each exp covers 1536 cols; tiles may span chunk boundaries
            # (exp doesn't care). PVs trail their act by one tile, and a
            # chunk's first PVs are deferred one act further so the
            # previous chunk's accumulator eviction can drain first.
            UH = N_QC * 2 * KB  # 128 units
            NTH = (UH + 2) // 3  # 43 tiles
            scs = {}
            acc = [None] * N_QC
            pend = []  # units whose act is emitted but PV is not
            n_acts = 0
            last_h = h == HPC - 1

            def u_decode(u):
                qc, r = divmod(u, 2 * KB)
                k, half = divmod(r, 2)
                return qc, k, half

            def emit_epilogue(qc):
                a1, a2 = acc[qc]
                last = last_h and qc == N_QC - 1
                # evict accumulators to SBUF; the next chunk's first PVs
                # are deferred one extra act so this drain can finish.
                nc.vector.tensor_copy(sa1[:, ts(qc, QC)], a1[:])
                if last:
                    nc.scalar.copy(sa2[:, ts(qc, QC)], a2[:])
                    # pre-load the Sqrt act table while the epilogue runs.
                    # The dummy must DEPEND on tail data (the sa2 eviction):
                    # with only an early dep it bypasses the queued exp acts
                    # via the wait-queue and runs at startup, so the real
                    # Sqrt pays a fresh table load on the critical chain.
                    nc.scalar.activation(
                        junk[:], sa2[:, qc * QC : qc * QC + 1], AF.Sqrt
                    )
                else:
                    nc.vector.tensor_copy(sa2[:, ts(qc, QC)], a2[:])

                rb1 = pep.tile([65, QC], f32)
                nc.gpsimd.partition_broadcast(
                    rb1[:], sa1[0:1, ts(qc, QC)], channels=65
                )
                rb2 = pep.tile([65, QC], f32)
                if last:
                    rb2_ps = pacc.tile([65, QC], f32, tag="a1")
                    nc.tensor.matmul(
                        rb2_ps[:],
                        lhsT=ones65[:],
                        rhs=sa2[0:1, ts(qc, QC)],
                        start=True,
                        stop=True,
                    )
                    nc.vector.reciprocal_approx_fast(rb2[:], rb2_ps[:])
                else:
                    nc.gpsimd.partition_broadcast(
                        rb2[:], sa2[0:1, ts(qc, QC)], channels=65
                    )
                    nc.vector.reciprocal_approx_fast(rb2[:], rb2[:])
                nc.vector.reciprocal_approx_fast(rb1[:], rb1[:])
                t1 = pep.tile([65, QC], f32)
                nc.vector.tensor_mul(t1[:], sa1[:, ts(qc, QC)], rb1[:])
                t2 = pep.tile([65, QC], f32)
                nc.vector.tensor_mul(t2[:], sa2[:, ts(qc, QC)], rb2[:])
                # outc = t1 - lam * t2  (row 0: lam-col is 1.0 -> exact 0)
                nc.vector.ln_bwd_dx(
                    outc[:, ts(qc, QC)],
                    dy=t1[:],
                    x_hat=t2[:],
                    mean_dyx=gbs[:, 2:3],
                    mean_dy=0.0,
                    scale=1.0,
                )
                nc.vector.bn_stats(st[:, qc, :], outc[:, ts(qc, QC)])

            def flush_pvs():
                # Emit PVs for pending units. A chunk's first units are
                # held until one act beyond the tile that contains them.
                while pend:
                    u = pend[0]
                    qc, k, half = u_decode(u)
                    # normal lag: two acts beyond the unit's own tile, so
                    # in the in-order PE queue the NEXT tile's QKs precede
                    # these PVs (which block on the previous act's exp +
                    # 100ns sem propagation) — the following act's QK
                    # dependency then resolves well before the act engine
                    # is free, instead of ~150ns late. Chunk-first units:
                    # one further, so the previous chunk's accumulator
                    # eviction can drain.
                    req = u // 3 + 3 + (1 if k == 0 else 0)
                    if n_acts < req:
                        break
                    if acc[qc] is None:
                        acc[qc] = (
                            pacc.tile([65, QC], f32, tag="a1", name="a1"),
                            pacc.tile([65, QC], f32, tag="a2", name="a2"),
                        )
                    t, i = divmod(u, 3)
                    e = scs[t][1]
                    nc.tensor.matmul(
                        acc[qc][half][:],
                        lhsT=vs[:, ts(k, 65)],
                        rhs=e[:, i * QC : (i + 1) * QC],
                        start=(k == 0),
                        stop=(k == KB - 1),
                    )
                    pend.pop(0)
                    if k == KB - 1 and half == 1:
                        emit_epilogue(qc)
                        acc[qc] = None

            next_act = 0
            for u in range(UH):
                qc, k, half = u_decode(u)
                t, off = divmod(u, 3)
                if off == 0:
                    scs[t] = (
                        psc.tile([128, 3 * QC], f32, tag="sc", name="sc_t"),
                        pe.tile([128, 3 * QC], f16, name="e_t"),
                    )
                ksk = ksh[k // 8][:, ts(k % 8, 128)]
                nc.tensor.matmul(
                    scs[t][0][:, off * QC : (off + 1) * QC],
                    lhsT=ksk[64 * half : 64 * half + 64, :],
                    rhs=qsh[qc][64 * half : 64 * half + 64, :],
                    start=True,
                    stop=True,
                )
                # after three full QK pairs (acts t0/t1 covered), emit the
                # previous head's deferred tail
                if half == 1 and u == 5 and prev_tail[0] is not None:
                    prev_tail[0]()
                    prev_tail[0] = None
                # prefetch the next head's tensors mid-stream, clear of
                # both this head's loads and its finalize out-DMAs; only
                # between QK pairs so the pair stays PE-adjacent
                if half == 1 and u == UH // 2 + 1 and h + 1 < HPC:
                    nxt_loads = emit_loads(h + 1)
                if half == 1:
                    while next_act < NTH and min(3 * next_act + 2, UH - 1) <= u:
                        ta = next_act
                        t_last = min(3 * ta + 2, UH - 1)
                        sc, e = scs[ta]
                        w = (t_last - 3 * ta + 1) * QC
                        nc.scalar.activation(
                            e[:, 0:w], sc[:, 0:w], AF.Exp, scale=SCALE
                        )
                        n_acts += 1
                        pend.extend(range(3 * ta, t_last + 1))
                        next_act += 1
                        flush_pvs()
            flush_pvs()

            def drain_and_finalize():
                while pend:
                    u = pend.pop(0)
                    qc, k, half = u_decode(u)
                    if acc[qc] is None:
                        acc[qc] = (
                            pacc.tile([65, QC], f32, tag="a1", name="a1"),
                            pacc.tile([65, QC], f32, tag="a2", name="a2"),
                        )
                    t, i = divmod(u, 3)
                    e = scs[t][1]
                    nc.tensor.matmul(
                        acc[qc][half][:],
                        lhsT=vs[:, ts(k, 65)],
                        rhs=e[:, i * QC : (i + 1) * QC],
                        start=(k == 0),
                        stop=(k == KB - 1),
                    )
                    if k == KB - 1 and half == 1:
                        emit_epilogue(qc)
                        acc[qc] = None

                # ---- head finalize (partition 0 rows: harmless zeros) ----
                mv = pst.tile([65, 2], f32)
                nc.vector.bn_aggr(mv[:], st[:])
                s2 = pst.tile([65, 2], f32)
                nc.vector.tensor_copy(s2[:, 0:1], mv[:, 0:1])
                # E[x^2]_p = var_p + mean_p^2
                nc.vector.tensor_scalar(
                    out=s2[:, 1:2],
                    in0=mv[:, 0:1],
                    scalar1=mv[:, 0:1],
                    scalar2=mv[:, 1:2],
                    op0=OP.mult,
                    op1=OP.add,
                )
                tot = pst.tile([65, 2], f32)
                nc.gpsimd.partition_all_reduce(
                    tot[:], s2[:], channels=65, reduce_op=bass_isa.ReduceOp.add
                )
                # tot = sums over partitions of per-partition (mean, E[x^2])
                # over 2048 elements; rows 1-64 carry signal -> /64.
                mu = pst.tile([65, 1], f32)
                nc.vector.tensor_scalar_mul(mu[:], tot[:, 0:1], 1.0 / 64.0)
                # veps = (tot1 - tot0*mu - (-64*eps))/64 = var + eps, fused
                # into one ln_bwd_dx pass (tot0*mu = 64*mu^2).
                veps = pst.tile([65, 1], f32)
                nc.vector.ln_bwd_dx(
                    veps[:],
                    dy=tot[:, 1:2],
                    x_hat=tot[:, 0:1],
                    mean_dyx=mu[:],
                    mean_dy=-64.0 * EPS,
                    scale=1.0 / 64.0,
                )
                if last_h:
                    # rstd = sqrt(1/veps): fast DVE reciprocal + ScalarE
                    # sqrt (table pre-loaded right after the last exp act;
                    # ScalarE is idle in the tail).
                    rv = pst.tile([65, 1], f32)
                    nc.vector.reciprocal_approx_fast(rv[:], veps[:])
                    rstd = pst.tile([65, 1], f32)
                    nc.scalar.activation(rstd[:], rv[:], AF.Sqrt)
                    cur = rstd[:]
                else:
                    # rstd = Quake-rsqrt on DVE (bitcast + Newton) so
                    # ScalarE stays on the exp table mid-stream.
                    ish = pst.tile([65, 1], i32)
                    nc.vector.tensor_scalar(
                        out=ish[:],
                        in0=veps[:].bitcast(i32),
                        scalar1=1,
                        scalar2=None,
                        op0=OP.logical_shift_right,
                    )
                    iy = pst.tile([65, 1], i32)
                    nc.vector.tensor_sub(iy[:], magic[:], ish[:])
                    vh = pst.tile([65, 1], f32)
                    nc.vector.tensor_scalar_mul(vh[:], veps[:], -0.5)
                    cur = iy[:].bitcast(f32)
                    for it in range(2):
                        aa = pst.tile([65, 1], f32, tag=f"nr_a{it}")
                        nc.vector.tensor_mul(aa[:], cur, cur)
                        bb = pst.tile([65, 1], f32, tag=f"nr_b{it}")
                        nc.vector.tensor_scalar(
                            out=bb[:], in0=aa[:], scalar1=vh[:], scalar2=1.5,
                            op0=OP.mult, op1=OP.add,
                        )
                        nxt = pst.tile([65, 1], f32, tag=f"nr_y{it}")
                        nc.vector.tensor_tensor(
                            out=nxt[:], in0=bb[:], in1=cur, op=OP.mult
                        )
                        cur = nxt[:]
                sg = pst.tile([65, 1], f32)
                nc.vector.tensor_tensor(
                    out=sg[:], in0=cur, in1=gbs[:, 0:1], op=OP.mult
                )
                tb = pst.tile([65, 1], f32)
                ms = pst.tile([65, 1], f32)
                nc.vector.tensor_scalar(
                    out=ms[:], in0=mu[:], scalar1=sg[:], scalar2=None, op0=OP.mult
                )
                nc.vector.tensor_sub(tb[:], gbs[:, 1:2], ms[:])
                # final affine, in pieces so each piece's output DMA
                # overlaps the next piece's apply. In the exposed tail
                # (last head) 8 narrow pieces run interleaved on
                # ScalarE/DVE and the DMAs rotate over three queues
                # (scalar takes the last two, after its affine work);
                # mid-stream heads stay off ScalarE.
                n_pieces = 8 if last_h else 2
                outf = pout.tile([65, S], f32)
                dmaq = [nc.sync, nc.gpsimd, nc.sync, nc.gpsimd,
                        nc.sync, nc.gpsimd, nc.scalar, nc.scalar]
                for piece in range(n_pieces):
                    sl = slice(
                        piece * (S // n_pieces), (piece + 1) * (S // n_pieces)
                    )
                    if last_h and piece % 2 == 0:
                        nc.scalar.activation(
                            outf[:, sl], outc[:, sl], AF.Identity,
                            bias=tb[:], scale=sg[:],
                        )
                    else:
                        nc.vector.tensor_scalar(
                            out=outf[:, sl],
                            in0=outc[:, sl],
                            scalar1=sg[:],
                            scalar2=tb[:],
                            op0=OP.mult,
                            op1=OP.add,
                        )
                    if last_h:
                        dmaq[piece].dma_start(outT[h, :, sl], outf[1:65, sl])
                    else:
                        nc.sync.dma_start(outT[h, :, sl], outf[1:65, sl])

            prev_tail[0] = drain_and_finalize
            return nxt_loads

        lds = emit_loads(0, startup=True)
        for h in range(HPC):
            lds = run_head(h, lds)
        prev_tail[0]()

    nc.compile()
    return nc


def _get_nc():
    if "nc" not in _CACHE:
        _CACHE["nc"] = _build_nc()
    return _CACHE["nc"]


def _host_prep(q, k, v, lq1, lq2, lk1, lk2, gamma, beta):
    """Build per-core input maps."""
    q = np.asarray(q, dtype=np.float32)
    k = np.asarray(k, dtype=np.float32)
    v = np.asarray(v, dtype=np.float32)
    lam = float(
        np.exp(np.float32(np.dot(lq1, lk1)))
        - np.exp(np.float32(np.dot(lq2, lk2)))
        + LAMBDA_INIT
    )
    g2 = (np.asarray(gamma, np.float32) * (1.0 - LAMBDA_INIT)).reshape(H, D)
    b2 = (np.asarray(beta, np.float32) * (1.0 - LAMBDA_INIT)).reshape(H, D)

    in_maps = []
    for c in range(N_CORES):
        heads = range(c * HPC, (c + 1) * HPC)
        qTa = np.empty((HPC, 128, S), np.float16)
        kTa = np.empty((HPC, 128, S), np.float16)
        vpa = np.empty((HPC, 128, KB * 65), np.float16)
        gba = np.empty((HPC, 64, 3), np.float32)
        for i, hh in enumerate(heads):
            qTa[i] = q[0, hh].T.astype(np.float16)
            kTa[i] = k[0, hh].T.astype(np.float16)
            vh = v[0, hh]  # [S, 64]
            v1 = np.concatenate([np.ones((S, 1), np.float32), vh], axis=1)
            # SBUF image: [partition(key within block), kblock*65 + col]
            vpa[i] = (
                v1.reshape(KB, 128, 65).transpose(1, 0, 2).reshape(128, KB * 65)
            ).astype(np.float16)
            gba[i, :, 0] = g2[hh]
            gba[i, :, 1] = b2[hh]
            gba[i, :, 2] = lam
        in_maps.append({"qT": qTa, "kT": kTa, "vp": vpa, "gb": gba})
    return in_maps


def kernel(q, k, v, lq1, lq2, lk1, lk2, gamma, beta, _trace=False, _tmpdir=None):
    from concourse.bass_utils import run_bass_kernel_spmd

    nc = _get_nc()
    in_maps = _host_prep(q, k, v, lq1, lq2, lk1, lk2, gamma, beta)
    res = run_bass_kernel_spmd(
        nc,
        in_maps,
        core_ids=list(range(N_CORES)),
        trace=_trace,
        tmpdir=_tmpdir,
    )
    out = np.empty((B, H, S, D), np.float32)
    for c in range(N_CORES):
        outT = res.results[c]["outT"]  # [HPC, 64, S]
        for i in range(HPC):
            out[0, c * HPC + i] = outT[i].T
    if _trace:
        _CACHE["last_results"] = res
    return out



# revision 12
# speedup vs baseline: 1.0911x; 1.0011x over previous
"""Differential attention (two-softmax diff + GroupNorm) on 8 TRN2 cores.

Sharding: 16 heads / 8 cores = 2 heads per core (head-parallel, no
collectives). GroupNorm stats are per-(batch, head) so each core is fully
independent.

Device layout choices (host prepares everything):
  - Q, K per head are host-transposed to [128(d), 2048(s)] fp16: partitions
    0-63 hold half-1 (q1/k1), partitions 64-127 hold half-2. QK^T then
    contracts over the partition dim directly, producing transposed score
    blocks S^T[key, query] in PSUM (fp32). The two 64-contraction QK
    matmuls issue back-to-back and the PE runs them concurrently on row
    groups h0/h64.
  - V per head is prefixed with a ones column (V' = [1 | V], 65 cols, fp16)
    and pre-arranged into the SBUF image [128(key of block), 16*65]: the PV
    matmul (lhsT = V'[kblk], rhs = exp(S^T)[kblk]) then yields the softmax
    denominator on partition 0 and the numerator on partitions 1-64 in one
    accumulation group. BOTH halves share the same V' stationary (w1*V and
    lam*w2*V use the same V); lam is applied in the per-chunk epilogue via
    the fused LN_BWD_DX DVE op (out = t1 - lam*t2).
  - Output stays in [d, q] layout on device; the host transposes it back.

fp16 is used on every matmul path: it streams through the PE at ~1
cycle/column with the same 10-bit mantissa class as tf32. exp() runs on
ScalarE straight out of PSUM, writing fp16.

Main loop per (head, 512-query chunk): 16 key blocks of
  QK matmul pair -> exp on ScalarE (PSUM -> SBUF) -> PV pair accumulate,
then a per-chunk epilogue slice (denominator broadcast on GpSimd, divide +
combine + bn_stats on DVE) that hides under later chunks' main loop.
ScalarE's exp stream is the pacing engine (~1.1us per key block); the PE
fits just beneath it, so no warm-up spinner is used (the HAM clock gate
settles by itself and junk matmuls only delay the first real block).

Tail: rstd = sqrt(reciprocal_approx_fast(var+eps)) with the Sqrt act
table pre-loaded via a dummy activation right after the last exp, and the
final affine + output DMA run in 4 interleaved pieces.
"""

import math

import numpy as np

B, H, S, D = 1, 16, 2048, 64
N_CORES = 8
HPC = H // N_CORES  # heads per core
QC = 512            # query-chunk width (PSUM bank budget)
N_QC = S // QC
KB = S // 128       # key blocks of 128
LAMBDA_INIT = 0.8
EPS = 1e-5
SCALE = 1.0 / math.sqrt(D)
N_WARMUP_MM = 10

_CACHE = {}


def _build_nc():
    from contextlib import ExitStack

    import concourse.bacc as bacc
    import concourse.bass as bass
    import concourse.tile as tile
    from concourse import bass_isa, mybir

    f32 = mybir.dt.float32
    f16 = mybir.dt.float16
    i32 = mybir.dt.int32
    AF = mybir.ActivationFunctionType
    OP = mybir.AluOpType
    ts = bass.ts

    nc = bacc.Bacc("TRN2", target_bir_lowering=False, debug=False)

    qT = nc.dram_tensor("qT", [HPC, 128, S], f16, kind="ExternalInput").ap()
    kT = nc.dram_tensor("kT", [HPC, 128, S], f16, kind="ExternalInput").ap()
    vp = nc.dram_tensor("vp", [HPC, 128, KB * 65], f16, kind="ExternalInput").ap()
    # per-head (gamma', beta', lam) columns; row 0 of the lam column is 1.0
    gb = nc.dram_tensor("gb", [HPC, 64, 3], f32, kind="ExternalInput").ap()
    outT = nc.dram_tensor("outT", [HPC, 64, S], f32, kind="ExternalOutput").ap()

    with tile.TileContext(nc) as tc, ExitStack() as ctx:
        pq = ctx.enter_context(tc.tile_pool(name="pq", bufs=2))
        pk = ctx.enter_context(tc.tile_pool(name="pk", bufs=2))
        pv = ctx.enter_context(tc.tile_pool(name="pv", bufs=2))
        pe = ctx.enter_context(tc.tile_pool(name="pe", bufs=4))
        psa = ctx.enter_context(tc.tile_pool(name="psa", bufs=2))
        pep = ctx.enter_context(tc.tile_pool(name="pep", bufs=2))
        pout = ctx.enter_context(tc.tile_pool(name="pout", bufs=2))
        pst = ctx.enter_context(tc.tile_pool(name="pst", bufs=2))
        psingle = ctx.enter_context(tc.tile_pool(name="psingle", bufs=1))
        psc = ctx.enter_context(tc.tile_pool(name="psc", bufs=2, space="PSUM"))
        pacc = ctx.enter_context(tc.tile_pool(name="pacc", bufs=1, space="PSUM"))

        eps_t = psingle.tile([65, 1], f32)
        nc.vector.memset(eps_t, EPS)
        ones65 = psingle.tile([1, 65], f32)
        nc.vector.memset(ones65, 1.0)
        junk = psingle.tile([65, 1], f32)
        nc.vector.memset(junk, 1.0)
        magic = psingle.tile([65, 1], i32)
        nc.vector.memset(magic, 0x5F3759DF)

        # PE warm-up: ~24 tiny back-to-back matmuls flip the HAM clock
        # gate toward 8/8 while the first head's DMAs are in flight. The
        # warm-up accumulator borrows the a1 slot; the first chunk's a1
        # allocation simply waits for the last warm-up matmul.
        wu_w = psingle.tile([128, 128], f16)
        nc.vector.memset(wu_w, 0.0)
        wu_ps = pacc.tile([128, 128], f32, tag="a1")
        for _ in range(N_WARMUP_MM):
            nc.tensor.matmul(
                wu_ps[:], lhsT=wu_w[:], rhs=wu_w[:], start=True, stop=True
            )

        def emit_loads(h, startup=False):
            """DMA in head h's tensors. At startup the loads spread over
            three queues (sync: K, gpsimd: Q + gb, vector: V') so the
            transfers run in parallel and the first key blocks' data
            (K[0:256], Q chunk 0) leads each queue; the mid-stream
            prefetch for the next head stays on the Sync queue (it is
            fully hidden under the exp stream)."""
            ksh = [pk.tile([128, S // 2], f16, tag=f"ks{j}", name="ks") for j in range(2)]
            qsh = [pq.tile([128, QC], f16, tag=f"qs{j}", name="qs") for j in range(N_QC)]
            vs = pv.tile([128, KB * 65], f16, tag="v")
            vw = KB * 65 // 4
            if startup:
                # Per-queue transfer rate is only ~34GB/s (1KB lines), so
                # the plan splits the critical first tensors (K[0:256] +
                # q0) three ways and then pipelines each queue in
                # consumption order (the gpsimd queue frees ~1us before
                # sync/scalar, so it leads with q0's left half):
                gbs = pst.tile([65, 3], f32, tag="gbs")
                nc.vector.memset(gbs[0:1, :], 0.0)
                nc.vector.memset(gbs[0:1, 2:3], 1.0)
                nc.gpsimd.dma_start(qsh[0][:, 0:256], qT[h, :, 0:256])
                nc.gpsimd.dma_start(gbs[1:65, :], gb[h])
                nc.scalar.dma_start(ksh[0][:, 0:256], kT[h, :, 0:256])
                nc.sync.dma_start(qsh[0][:, 256:QC], qT[h, :, 256:QC])
                nc.gpsimd.dma_start(vs[:, 0:vw], vp[h, :, 0:vw])
                nc.sync.dma_start(ksh[0][:, 256:512], kT[h, :, 256:512])
                nc.gpsimd.dma_start(ksh[1][:, 0:512], kT[h, :, 1024:1536])
                nc.sync.dma_start(ksh[0][:, 512:768], kT[h, :, 512:768])
                nc.gpsimd.dma_start(vs[:, vw : 2 * vw], vp[h, :, vw : 2 * vw])
                nc.sync.dma_start(ksh[0][:, 768:1024], kT[h, :, 768:1024])
                nc.gpsimd.dma_start(qsh[1][:], qT[h, :, QC : 2 * QC])
                nc.sync.dma_start(ksh[1][:, 512:1024], kT[h, :, 1536:2048])
                nc.gpsimd.dma_start(vs[:, 2 * vw : 3 * vw], vp[h, :, 2 * vw : 3 * vw])
                nc.gpsimd.dma_start(vs[:, 3 * vw :], vp[h, :, 3 * vw :])
                nc.gpsimd.dma_start(qsh[2][:], qT[h, :, 2 * QC : 3 * QC])
                nc.gpsimd.dma_start(qsh[3][:], qT[h, :, 3 * QC : 4 * QC])
                return ksh, qsh, vs, gbs
            else:
                nc.sync.dma_start(ksh[0][:, 0:256], kT[h, :, 0:256])
                nc.sync.dma_start(qsh[0][:], qT[h, :, 0:QC])
                nc.sync.dma_start(ksh[0][:, 256 : S // 2], kT[h, :, 256 : S // 2])
                nc.sync.dma_start(qsh[1][:], qT[h, :, QC : 2 * QC])
                nc.sync.dma_start(vs[:, 0:vw], vp[h, :, 0:vw])
                nc.sync.dma_start(vs[:, vw : 2 * vw], vp[h, :, vw : 2 * vw])
                nc.sync.dma_start(ksh[1][:], kT[h, :, S // 2 : S])
                nc.sync.dma_start(vs[:, 2 * vw : 3 * vw], vp[h, :, 2 * vw : 3 * vw])
                nc.sync.dma_start(vs[:, 3 * vw :], vp[h, :, 3 * vw :])
                for j in range(2, N_QC):
                    nc.sync.dma_start(qsh[j][:], qT[h, :, j * QC : (j + 1) * QC])
            gbs = pst.tile([65, 3], f32, tag="gbs")
            nc.vector.memset(gbs[0:1, :], 0.0)
            nc.vector.memset(gbs[0:1, 2:3], 1.0)
            nc.gpsimd.dma_start(gbs[1:65, :], gb[h])
            return ksh, qsh, vs, gbs

        # Deferred per-head tail: the previous head's last-tile PVs,
        # epilogue and finalize are emitted a few QK pairs into the NEXT
        # head's stream, so they don't sit between the last act and the
        # next head's first QKs in the in-order PE queue (same fix as the
        # chunk-boundary PV deferral, applied at the head seam).
        prev_tail = [None]

        def run_head(h, loads):
            ksh, qsh, vs, gbs = loads
            nxt_loads = None
            last_h = h == HPC - 1

            # Query-chunk layout. The last head tapers to two 256-wide
            # chunks at the end so the final (exposed) epilogue's DVE
            # chain is half length; mid-stream epilogues hide under the
            # exp stream either way.
            cws = [QC] * N_QC  # mixed tail chunks tripped a HW error; see below
            NCH = len(cws)
            css = [sum(cws[:i]) for i in range(NCH)]

            # Units: one (chunk, key-block, half) score block of cw
            # columns; chunks outer, then k, then half.
            u_ci, u_k, u_half = [], [], []
            for ci in range(NCH):
                for k in range(KB):
                    for half in (0, 1):
                        u_ci.append(ci)
                        u_k.append(k)
                        u_half.append(half)
            UH = len(u_ci)

            # Tiles: greedy-pack units into <=1536 score columns (3 PSUM
            # banks); each exp act covers one tile. Tiles may span chunk
            # boundaries; unit offsets stay bank-aligned because
            # 256-wide units always come in pairs.
            u_tile, u_off, tiles = [], [], []
            cur_lo, cur_w = 0, 0
            for u in range(UH):
                cw = cws[u_ci[u]]
                if cur_w + cw > 3 * QC:
                    tiles.append((cur_lo, u - 1, cur_w))
                    cur_lo, cur_w = u, 0
                u_tile.append(len(tiles))
                u_off.append(cur_w)
                cur_w += cw
            tiles.append((cur_lo, UH - 1, cur_w))
            NT = len(tiles)

            # [denominator(row 0) | numerator(rows 1-64)] x all queries
            sa1 = psa.tile([65, S], f32)
            sa2 = psa.tile([65, S], f32)
            outc = pout.tile([65, S], f32)
            st = pst.tile([65, 5, 6], f32, tag="st")

            scs = {}
            acc = [None] * NCH
            pend = []  # units whose act is emitted but PV is not
            n_acts = 0

            def emit_epilogue(ci):
                a1, a2 = acc[ci]
                cs0, cw = css[ci], cws[ci]
                sl = slice(cs0, cs0 + cw)
                last = last_h and ci == NCH - 1
                # evict accumulators to SBUF; the next chunk's first PVs
                # are deferred one extra act so this drain can finish.
                nc.vector.tensor_copy(sa1[:, sl], a1[:, :cw])
                if last:
                    nc.scalar.copy(sa2[:, sl], a2[:, :cw])
                    # pre-load the Sqrt act table while the epilogue runs.
                    # The dummy must DEPEND on tail data (the sa2 eviction):
                    # with only an early dep it bypasses the queued exp acts
                    # via the wait-queue and runs at startup, so the real
                    # Sqrt pays a fresh table load on the critical chain.
                    nc.scalar.activation(
                        junk[:], sa2[:, cs0 : cs0 + 1], AF.Sqrt
                    )
                else:
                    nc.vector.tensor_copy(sa2[:, sl], a2[:, :cw])

                rb1 = pep.tile([65, QC], f32, tag="rb1")
                nc.gpsimd.partition_broadcast(
                    rb1[:, :cw], sa1[0:1, sl], channels=65
                )
                rb2 = pep.tile([65, QC], f32, tag="rb2")
                if last:
                    rb2_ps = pacc.tile([65, QC], f32, tag="a1")
                    nc.tensor.matmul(
                        rb2_ps[:, :cw],
                        lhsT=ones65[:],
                        rhs=sa2[0:1, sl],
                        start=True,
                        stop=True,
                    )
                    nc.vector.reciprocal_approx_fast(rb2[:, :cw], rb2_ps[:, :cw])
                else:
                    nc.gpsimd.partition_broadcast(
                        rb2[:, :cw], sa2[0:1, sl], channels=65
                    )
                    nc.vector.reciprocal_approx_fast(rb2[:, :cw], rb2[:, :cw])
                nc.vector.reciprocal_approx_fast(rb1[:, :cw], rb1[:, :cw])
                t1 = pep.tile([65, QC], f32, tag="t1")
                nc.vector.tensor_mul(t1[:, :cw], sa1[:, sl], rb1[:, :cw])
                t2 = pep.tile([65, QC], f32, tag="t2")
                nc.vector.tensor_mul(t2[:, :cw], sa2[:, sl], rb2[:, :cw])
                # outc = t1 - lam * t2  (row 0: lam-col is 1.0 -> exact 0)
                nc.vector.ln_bwd_dx(
                    outc[:, sl],
                    dy=t1[:, :cw],
                    x_hat=t2[:, :cw],
                    mean_dyx=gbs[:, 2:3],
                    mean_dy=0.0,
                    scale=1.0,
                )
                nc.vector.bn_stats(st[:, ci, :], outc[:, sl])

            def emit_pv(u):
                ci, k, half = u_ci[u], u_k[u], u_half[u]
                cw = cws[ci]
                if acc[ci] is None:
                    acc[ci] = (
                        pacc.tile([65, QC], f32, tag="a1", name="a1"),
                        pacc.tile([65, QC], f32, tag="a2", name="a2"),
                    )
                e = scs[u_tile[u]][1]
                nc.tensor.matmul(
                    acc[ci][half][:, :cw],
                    lhsT=vs[:, ts(k, 65)],
                    rhs=e[:, u_off[u] : u_off[u] + cw],
                    start=(k == 0),
                    stop=(k == KB - 1),
                )
                if k == KB - 1 and half == 1:
                    emit_epilogue(ci)
                    acc[ci] = None

            def flush_pvs():
                # Emit PVs for pending units. Normal lag: two acts beyond
                # the unit's own tile, so in the in-order PE queue the
                # NEXT tile's QKs precede these PVs (which block on the
                # previous act's exp + 100ns sem propagation) — the
                # following act's QK dependency then resolves well before
                # the act engine is free, instead of ~150ns late.
                # Chunk-first units: one act further, so the previous
                # chunk's accumulator eviction can drain.
                while pend:
                    u = pend[0]
                    req = u_tile[u] + 3 + (1 if u_k[u] == 0 else 0)
                    if n_acts < req:
                        break
                    pend.pop(0)
                    emit_pv(u)

            next_act = 0
            for u in range(UH):
                ci, k, half = u_ci[u], u_k[u], u_half[u]
                t = u_tile[u]
                cw = cws[ci]
                if u_off[u] == 0:
                    scs[t] = (
                        psc.tile([128, 3 * QC], f32, tag="sc", name="sc_t"),
                        pe.tile([128, 3 * QC], f16, name="e_t"),
                    )
                ksk = ksh[k // 8][:, ts(k % 8, 128)]
                cs0 = css[ci]
                qt = qsh[cs0 // QC]
                qo = cs0 % QC
                nc.tensor.matmul(
                    scs[t][0][:, u_off[u] : u_off[u] + cw],
                    lhsT=ksk[64 * half : 64 * half + 64, :],
                    rhs=qt[64 * half : 64 * half + 64, qo : qo + cw],
                    start=True,
                    stop=True,
                )
                # after three full QK pairs (acts t0/t1 covered), emit the
                # previous head's deferred tail
                if half == 1 and u == 5 and prev_tail[0] is not None:
                    prev_tail[0]()
                    prev_tail[0] = None
                # prefetch the next head's tensors mid-stream, clear of
                # both this head's loads and its finalize out-DMAs; only
                # between QK pairs so the pair stays PE-adjacent
                if half == 1 and u == UH // 2 + 1 and h + 1 < HPC:
                    nxt_loads = emit_loads(h + 1)
                if half == 1:
                    while next_act < NT and tiles[next_act][1] <= u:
                        lo, hi, w = tiles[next_act]
                        sc, e = scs[next_act]
                        nc.scalar.activation(
                            e[:, 0:w], sc[:, 0:w], AF.Exp, scale=SCALE
                        )
                        n_acts += 1
                        pend.extend(range(lo, hi + 1))
                        next_act += 1
                        flush_pvs()
            flush_pvs()

            def drain_and_finalize():
                while pend:
                    emit_pv(pend.pop(0))

                # ---- head finalize (partition 0 rows: harmless zeros) ----
                mv = pst.tile([65, 2], f32)
                nc.vector.bn_aggr(mv[:], st[:, :NCH, :])
                s2 = pst.tile([65, 2], f32)
                nc.vector.tensor_copy(s2[:, 0:1], mv[:, 0:1])
                # E[x^2]_p = var_p + mean_p^2
                nc.vector.tensor_scalar(
                    out=s2[:, 1:2],
                    in0=mv[:, 0:1],
                    scalar1=mv[:, 0:1],
                    scalar2=mv[:, 1:2],
                    op0=OP.mult,
                    op1=OP.add,
                )
                tot = pst.tile([65, 2], f32)
                nc.gpsimd.partition_all_reduce(
                    tot[:], s2[:], channels=65, reduce_op=bass_isa.ReduceOp.add
                )
                # tot = sums over partitions of per-partition (mean, E[x^2])
                # over 2048 elements; rows 1-64 carry signal -> /64.
                mu = pst.tile([65, 1], f32)
                nc.vector.tensor_scalar_mul(mu[:], tot[:, 0:1], 1.0 / 64.0)
                # veps = (tot1 - tot0*mu - (-64*eps))/64 = var + eps, fused
                # into one ln_bwd_dx pass (tot0*mu = 64*mu^2).
                veps = pst.tile([65, 1], f32)
                nc.vector.ln_bwd_dx(
                    veps[:],
                    dy=tot[:, 1:2],
                    x_hat=tot[:, 0:1],
                    mean_dyx=mu[:],
                    mean_dy=-64.0 * EPS,
                    scale=1.0 / 64.0,
                )
                if last_h:
                    # rstd = sqrt(1/veps): fast DVE reciprocal + ScalarE
                    # sqrt (table pre-loaded right after the last exp act;
                    # ScalarE is idle in the tail).
                    rv = pst.tile([65, 1], f32)
                    nc.vector.reciprocal_approx_fast(rv[:], veps[:])
                    rstd = pst.tile([65, 1], f32)
                    nc.scalar.activation(rstd[:], rv[:], AF.Sqrt)
                    cur = rstd[:]
                else:
                    # rstd = Quake-rsqrt on DVE (bitcast + Newton) so
                    # ScalarE stays on the exp table mid-stream.
                    ish = pst.tile([65, 1], i32)
                    nc.vector.tensor_scalar(
                        out=ish[:],
                        in0=veps[:].bitcast(i32),
                        scalar1=1,
                        scalar2=None,
                        op0=OP.logical_shift_right,
                    )
                    iy = pst.tile([65, 1], i32)
                    nc.vector.tensor_sub(iy[:], magic[:], ish[:])
                    vh = pst.tile([65, 1], f32)
                    nc.vector.tensor_scalar_mul(vh[:], veps[:], -0.5)
                    cur = iy[:].bitcast(f32)
                    for it in range(2):
                        aa = pst.tile([65, 1], f32, tag=f"nr_a{it}")
                        nc.vector.tensor_mul(aa[:], cur, cur)
                        bb = pst.tile([65, 1], f32, tag=f"nr_b{it}")
                        nc.vector.tensor_scalar(
                            out=bb[:], in0=aa[:], scalar1=vh[:], scalar2=1.5,
                            op0=OP.mult, op1=OP.add,
                        )
                        nxt = pst.tile([65, 1], f32, tag=f"nr_y{it}")
                        nc.vector.tensor_tensor(
                            out=nxt[:], in0=bb[:], in1=cur, op=OP.mult
                        )
                        cur = nxt[:]
                sg = pst.tile([65, 1], f32)
                nc.vector.tensor_tensor(
                    out=sg[:], in0=cur, in1=gbs[:, 0:1], op=OP.mult
                )
                tb = pst.tile([65, 1], f32)
                ms = pst.tile([65, 1], f32)
                nc.vector.tensor_scalar(
                    out=ms[:], in0=mu[:], scalar1=sg[:], scalar2=None, op0=OP.mult
                )
                nc.vector.tensor_sub(tb[:], gbs[:, 1:2], ms[:])
                # final affine, in pieces so each piece's output DMA
                # overlaps the next piece's apply. In the exposed tail
                # (last head) pieces alternate ScalarE/DVE and the DMAs
                # rotate over three queues (scalar's DMA follows its own
                # affine piece on the same queue); mid-stream heads stay
                # off ScalarE.
                n_pieces = 4 if last_h else 2
                outf = pout.tile([65, S], f32)
                dmaq = [nc.sync, nc.gpsimd, nc.scalar, nc.gpsimd]
                for piece in range(n_pieces):
                    sl = slice(
                        piece * (S // n_pieces), (piece + 1) * (S // n_pieces)
                    )
                    if last_h and piece % 2 == 0:
                        nc.scalar.activation(
                            outf[:, sl], outc[:, sl], AF.Identity,
                            bias=tb[:], scale=sg[:],
                        )
                    else:
                        nc.vector.tensor_scalar(
                            out=outf[:, sl],
                            in0=outc[:, sl],
                            scalar1=sg[:],
                            scalar2=tb[:],
                            op0=OP.mult,
                            op1=OP.add,
                        )
                    if last_h:
                        dmaq[piece].dma_start(outT[h, :, sl], outf[1:65, sl])
                    else:
                        nc.sync.dma_start(outT[h, :, sl], outf[1:65, sl])

            prev_tail[0] = drain_and_finalize
            return nxt_loads

        lds = emit_loads(0, startup=True)
        for h in range(HPC):
            lds = run_head(h, lds)
        prev_tail[0]()

    nc.compile()
    return nc


def _get_nc():
    if "nc" not in _CACHE:
        _CACHE["nc"] = _build_nc()
    return _CACHE["nc"]


def _host_prep(q, k, v, lq1, lq2, lk1, lk2, gamma, beta):
    """Build per-core input maps."""
    q = np.asarray(q, dtype=np.float32)
    k = np.asarray(k, dtype=np.float32)
    v = np.asarray(v, dtype=np.float32)
    lam = float(
        np.exp(np.float32(np.dot(lq1, lk1)))
        - np.exp(np.float32(np.dot(lq2, lk2)))
        + LAMBDA_INIT
    )
    g2 = (np.asarray(gamma, np.float32) * (1.0 - LAMBDA_INIT)).reshape(H, D)
    b2 = (np.asarray(beta, np.float32) * (1.0 - LAMBDA_INIT)).reshape(H, D)

    in_maps = []
    for c in range(N_CORES):
        heads = range(c * HPC, (c + 1) * HPC)
        qTa = np.empty((HPC, 128, S), np.float16)
        kTa = np.empty((HPC, 128, S), np.float16)
        vpa = np.empty((HPC, 128, KB * 65), np.float16)
        gba = np.empty((HPC, 64, 3), np.float32)
        for i, hh in enumerate(heads):
            qTa[i] = q[0, hh].T.astype(np.float16)
            kTa[i] = k[0, hh].T.astype(np.float16)
            vh = v[0, hh]  # [S, 64]
            v1 = np.concatenate([np.ones((S, 1), np.float32), vh], axis=1)
            # SBUF image: [partition(key within block), kblock*65 + col]
            vpa[i] = (
                v1.reshape(KB, 128, 65).transpose(1, 0, 2).reshape(128, KB * 65)
            ).astype(np.float16)
            gba[i, :, 0] = g2[hh]
            gba[i, :, 1] = b2[hh]
            gba[i, :, 2] = lam
        in_maps.append({"qT": qTa, "kT": kTa, "vp": vpa, "gb": gba})
    return in_maps


def kernel(q, k, v, lq1, lq2, lk1, lk2, gamma, beta, _trace=False, _tmpdir=None):
    from concourse.bass_utils import run_bass_kernel_spmd

    nc = _get_nc()
    in_maps = _host_prep(q, k, v, lq1, lq2, lk1, lk2, gamma, beta)
    res = run_bass_kernel_spmd(
        nc,
        in_maps,
        core_ids=list(range(N_CORES)),
        trace=_trace,
        tmpdir=_tmpdir,
    )
    out = np.empty((B, H, S, D), np.float32)
    for c in range(N_CORES):
        outT = res.results[c]["outT"]  # [HPC, 64, S]
        for i in range(HPC):
            out[0, c * HPC + i] = outT[i].T
    if _trace:
        _CACHE["last_results"] = res
    return out



# revision 14
# speedup vs baseline: 1.1010x; 1.0091x over previous
"""Differential attention (two-softmax diff + GroupNorm) on 8 TRN2 cores.

Sharding: 16 heads / 8 cores = 2 heads per core (head-parallel, no
collectives). GroupNorm stats are per-(batch, head) so each core is fully
independent.

Device layout choices (host prepares everything):
  - Q, K per head are host-transposed to [128(d), 2048(s)] fp16: partitions
    0-63 hold half-1 (q1/k1), partitions 64-127 hold half-2. QK^T then
    contracts over the partition dim directly, producing transposed score
    blocks S^T[key, query] in PSUM (fp32). The two 64-contraction QK
    matmuls issue back-to-back and the PE runs them concurrently on row
    groups h0/h64.
  - V per head is prefixed with a ones column (V' = [1 | V], 65 cols, fp16)
    and pre-arranged into the SBUF image [128(key of block), 16*65]: the PV
    matmul (lhsT = V'[kblk], rhs = exp(S^T)[kblk]) then yields the softmax
    denominator on partition 0 and the numerator on partitions 1-64 in one
    accumulation group. BOTH halves share the same V' stationary (w1*V and
    lam*w2*V use the same V); lam is applied in the per-chunk epilogue via
    the fused LN_BWD_DX DVE op (out = t1 - lam*t2).
  - Output stays in [d, q] layout on device; the host transposes it back.

fp16 is used on every matmul path: it streams through the PE at ~1
cycle/column with the same 10-bit mantissa class as tf32. exp() runs on
ScalarE straight out of PSUM, writing fp16.

Main loop per (head, 512-query chunk): 16 key blocks of
  QK matmul pair -> exp on ScalarE (PSUM -> SBUF) -> PV pair accumulate,
then a per-chunk epilogue slice (denominator broadcast on GpSimd, divide +
combine + bn_stats on DVE) that hides under later chunks' main loop.
ScalarE's exp stream is the pacing engine (~1.1us per key block); the PE
fits just beneath it, so no warm-up spinner is used (the HAM clock gate
settles by itself and junk matmuls only delay the first real block).

Tail: rstd = sqrt(reciprocal_approx_fast(var+eps)) with the Sqrt act
table pre-loaded via a dummy activation right after the last exp, and the
final affine + output DMA run in 4 interleaved pieces.
"""

import math

import numpy as np

B, H, S, D = 1, 16, 2048, 64
N_CORES = 8
HPC = H // N_CORES  # heads per core
QC = 512            # query-chunk width (PSUM bank budget)
N_QC = S // QC
KB = S // 128       # key blocks of 128
LAMBDA_INIT = 0.8
EPS = 1e-5
SCALE = 1.0 / math.sqrt(D)
N_WARMUP_MM = 10

_CACHE = {}


def _build_nc():
    from contextlib import ExitStack

    import concourse.bacc as bacc
    import concourse.bass as bass
    import concourse.tile as tile
    from concourse import bass_isa, mybir

    f32 = mybir.dt.float32
    f16 = mybir.dt.float16
    i32 = mybir.dt.int32
    AF = mybir.ActivationFunctionType
    OP = mybir.AluOpType
    ts = bass.ts

    nc = bacc.Bacc("TRN2", target_bir_lowering=False, debug=False)

    qT = nc.dram_tensor("qT", [HPC, 128, S], f16, kind="ExternalInput").ap()
    kT = nc.dram_tensor("kT", [HPC, 128, S], f16, kind="ExternalInput").ap()
    vp = nc.dram_tensor("vp", [HPC, 128, KB * 65], f16, kind="ExternalInput").ap()
    # per-head (gamma', beta', lam) columns; row 0 of the lam column is 1.0
    gb = nc.dram_tensor("gb", [HPC, 64, 3], f32, kind="ExternalInput").ap()
    outT = nc.dram_tensor("outT", [HPC, 64, S], f32, kind="ExternalOutput").ap()

    with tile.TileContext(nc) as tc, ExitStack() as ctx:
        pq = ctx.enter_context(tc.tile_pool(name="pq", bufs=2))
        pk = ctx.enter_context(tc.tile_pool(name="pk", bufs=2))
        pv = ctx.enter_context(tc.tile_pool(name="pv", bufs=2))
        pe = ctx.enter_context(tc.tile_pool(name="pe", bufs=4))
        psa = ctx.enter_context(tc.tile_pool(name="psa", bufs=2))
        pep = ctx.enter_context(tc.tile_pool(name="pep", bufs=2))
        pout = ctx.enter_context(tc.tile_pool(name="pout", bufs=2))
        pst = ctx.enter_context(tc.tile_pool(name="pst", bufs=2))
        psingle = ctx.enter_context(tc.tile_pool(name="psingle", bufs=1))
        psc = ctx.enter_context(tc.tile_pool(name="psc", bufs=2, space="PSUM"))
        pacc = ctx.enter_context(tc.tile_pool(name="pacc", bufs=1, space="PSUM"))

        eps_t = psingle.tile([65, 1], f32)
        nc.vector.memset(eps_t, EPS)
        ones65 = psingle.tile([1, 65], f32)
        nc.vector.memset(ones65, 1.0)
        junk = psingle.tile([65, 1], f32)
        nc.vector.memset(junk, 1.0)
        magic = psingle.tile([65, 1], i32)
        nc.vector.memset(magic, 0x5F3759DF)

        # PE warm-up: ~24 tiny back-to-back matmuls flip the HAM clock
        # gate toward 8/8 while the first head's DMAs are in flight. The
        # warm-up accumulator borrows the a1 slot; the first chunk's a1
        # allocation simply waits for the last warm-up matmul.
        wu_w = psingle.tile([128, 128], f16)
        nc.vector.memset(wu_w, 0.0)
        wu_ps = pacc.tile([128, 128], f32, tag="a1")
        for _ in range(N_WARMUP_MM):
            nc.tensor.matmul(
                wu_ps[:], lhsT=wu_w[:], rhs=wu_w[:], start=True, stop=True
            )

        def emit_loads(h, startup=False):
            """DMA in head h's tensors. At startup the loads spread over
            three queues (sync: K, gpsimd: Q + gb, vector: V') so the
            transfers run in parallel and the first key blocks' data
            (K[0:256], Q chunk 0) leads each queue; the mid-stream
            prefetch for the next head stays on the Sync queue (it is
            fully hidden under the exp stream)."""
            ksh = [pk.tile([128, S // 2], f16, tag=f"ks{j}", name="ks") for j in range(2)]
            qsh = [pq.tile([128, QC], f16, tag=f"qs{j}", name="qs") for j in range(N_QC)]
            vs = pv.tile([128, KB * 65], f16, tag="v")
            vw = KB * 65 // 4
            if startup:
                # Per-queue transfer rate is only ~34GB/s (1KB lines), so
                # the plan splits the critical first tensors (K[0:256] +
                # q0) three ways and then pipelines each queue in
                # consumption order (the gpsimd queue frees ~1us before
                # sync/scalar, so it leads with q0's left half):
                gbs = pst.tile([65, 3], f32, tag="gbs")
                nc.vector.memset(gbs[0:1, :], 0.0)
                nc.vector.memset(gbs[0:1, 2:3], 1.0)
                nc.gpsimd.dma_start(qsh[0][:, 0:256], qT[h, :, 0:256])
                nc.gpsimd.dma_start(gbs[1:65, :], gb[h])
                nc.scalar.dma_start(ksh[0][:, 0:256], kT[h, :, 0:256])
                nc.sync.dma_start(qsh[0][:, 256:QC], qT[h, :, 256:QC])
                nc.gpsimd.dma_start(vs[:, 0:vw], vp[h, :, 0:vw])
                nc.sync.dma_start(ksh[0][:, 256:512], kT[h, :, 256:512])
                nc.gpsimd.dma_start(ksh[1][:, 0:512], kT[h, :, 1024:1536])
                nc.sync.dma_start(ksh[0][:, 512:768], kT[h, :, 512:768])
                nc.gpsimd.dma_start(vs[:, vw : 2 * vw], vp[h, :, vw : 2 * vw])
                nc.sync.dma_start(ksh[0][:, 768:1024], kT[h, :, 768:1024])
                nc.gpsimd.dma_start(qsh[1][:], qT[h, :, QC : 2 * QC])
                nc.sync.dma_start(ksh[1][:, 512:1024], kT[h, :, 1536:2048])
                nc.gpsimd.dma_start(vs[:, 2 * vw : 3 * vw], vp[h, :, 2 * vw : 3 * vw])
                nc.gpsimd.dma_start(vs[:, 3 * vw :], vp[h, :, 3 * vw :])
                nc.gpsimd.dma_start(qsh[2][:], qT[h, :, 2 * QC : 3 * QC])
                nc.gpsimd.dma_start(qsh[3][:], qT[h, :, 3 * QC : 4 * QC])
                return ksh, qsh, vs, gbs
            else:
                nc.sync.dma_start(ksh[0][:, 0:256], kT[h, :, 0:256])
                nc.sync.dma_start(qsh[0][:], qT[h, :, 0:QC])
                nc.sync.dma_start(ksh[0][:, 256 : S // 2], kT[h, :, 256 : S // 2])
                nc.sync.dma_start(qsh[1][:], qT[h, :, QC : 2 * QC])
                nc.sync.dma_start(vs[:, 0:vw], vp[h, :, 0:vw])
                nc.sync.dma_start(vs[:, vw : 2 * vw], vp[h, :, vw : 2 * vw])
                nc.sync.dma_start(ksh[1][:], kT[h, :, S // 2 : S])
                nc.sync.dma_start(vs[:, 2 * vw : 3 * vw], vp[h, :, 2 * vw : 3 * vw])
                nc.sync.dma_start(vs[:, 3 * vw :], vp[h, :, 3 * vw :])
                for j in range(2, N_QC):
                    nc.sync.dma_start(qsh[j][:], qT[h, :, j * QC : (j + 1) * QC])
            gbs = pst.tile([65, 3], f32, tag="gbs")
            nc.vector.memset(gbs[0:1, :], 0.0)
            nc.vector.memset(gbs[0:1, 2:3], 1.0)
            nc.gpsimd.dma_start(gbs[1:65, :], gb[h])
            return ksh, qsh, vs, gbs

        # Deferred per-head tail: the previous head's last-tile PVs,
        # epilogue and finalize are emitted a few QK pairs into the NEXT
        # head's stream, so they don't sit between the last act and the
        # next head's first QKs in the in-order PE queue (same fix as the
        # chunk-boundary PV deferral, applied at the head seam).
        prev_tail = [None]

        def run_head(h, loads):
            ksh, qsh, vs, gbs = loads
            nxt_loads = None
            last_h = h == HPC - 1

            # Query-chunk layout. The last head tapers to two 256-wide
            # chunks at the end so the final (exposed) epilogue's DVE
            # chain is half length; mid-stream epilogues hide under the
            # exp stream either way.
            cws = [512, 512, 512, 256, 256] if last_h else [QC] * N_QC
            NCH = len(cws)
            css = [sum(cws[:i]) for i in range(NCH)]

            # Units: one (chunk, key-block, half) score block of cw
            # columns; chunks outer, then k, then half.
            u_ci, u_k, u_half = [], [], []
            for ci in range(NCH):
                for k in range(KB):
                    for half in (0, 1):
                        u_ci.append(ci)
                        u_k.append(k)
                        u_half.append(half)
            UH = len(u_ci)

            # Tiles: pack units into <=1536 score columns (3 PSUM banks);
            # each exp act covers one tile. 512-wide units go 3 per tile
            # at natural offsets. 256-wide units go 6 per tile with
            # PERMUTED offsets so the column-bank sequence is 0,1,2,0,1,2:
            # the QK pair (h0/h64 row groups) runs concurrently on the PE
            # and two in-flight matmuls draining into the same PSUM bank
            # is a fatal collision — adjacent units must differ in bank.
            u_tile, u_off, tiles = [], [], []
            i = 0
            while i < UH:
                if cws[u_ci[i]] == 512:
                    j = i
                    while j < UH and j - i < 3 and cws[u_ci[j]] == 512:
                        j += 1
                    offs = [512 * t for t in range(j - i)]
                    w = 512 * (j - i)
                else:
                    j = i
                    while j < UH and j - i < 6 and cws[u_ci[j]] == 256:
                        j += 1
                    n = j - i  # always even (half pairs)
                    if n == 6:
                        offs = [0, 512, 1024, 256, 768, 1280]
                        w = 1536
                    else:
                        offs = [0, 512, 256, 768][:n]
                        w = 1024 if n == 4 else 768
                for t, u in enumerate(range(i, j)):
                    u_tile.append(len(tiles))
                    u_off.append(offs[t])
                tiles.append((i, j - 1, w))
                i = j
            NT = len(tiles)

            # [denominator(row 0) | numerator(rows 1-64)] x all queries
            sa1 = psa.tile([65, S], f32)
            sa2 = psa.tile([65, S], f32)
            outc = pout.tile([65, S], f32)
            st = pst.tile([65, 5, 6], f32, tag="st")

            scs = {}
            acc = [None] * NCH
            pend = []  # units whose act is emitted but PV is not
            n_acts = 0

            def emit_epilogue(ci):
                a1, a2 = acc[ci]
                cs0, cw = css[ci], cws[ci]
                sl = slice(cs0, cs0 + cw)
                last = last_h and ci == NCH - 1
                # evict accumulators to SBUF; the next chunk's first PVs
                # are deferred one extra act so this drain can finish.
                nc.vector.tensor_copy(sa1[:, sl], a1[:, :cw])
                if last:
                    nc.scalar.copy(sa2[:, sl], a2[:, :cw])
                    # pre-load the Sqrt act table while the epilogue runs.
                    # The dummy must DEPEND on tail data (the sa2 eviction):
                    # with only an early dep it bypasses the queued exp acts
                    # via the wait-queue and runs at startup, so the real
                    # Sqrt pays a fresh table load on the critical chain.
                    nc.scalar.activation(
                        junk[:], sa2[:, cs0 : cs0 + 1], AF.Sqrt
                    )
                else:
                    nc.vector.tensor_copy(sa2[:, sl], a2[:, :cw])

                rb1 = pep.tile([65, QC], f32, tag="rb1")
                nc.gpsimd.partition_broadcast(
                    rb1[:, :cw], sa1[0:1, sl], channels=65
                )
                rb2 = pep.tile([65, QC], f32, tag="rb2")
                if last:
                    rb2_ps = pacc.tile([65, QC], f32, tag="a1")
                    nc.tensor.matmul(
                        rb2_ps[:, :cw],
                        lhsT=ones65[:],
                        rhs=sa2[0:1, sl],
                        start=True,
                        stop=True,
                    )
                    nc.vector.reciprocal_approx_fast(rb2[:, :cw], rb2_ps[:, :cw])
                else:
                    nc.gpsimd.partition_broadcast(
                        rb2[:, :cw], sa2[0:1, sl], channels=65
                    )
                    nc.vector.reciprocal_approx_fast(rb2[:, :cw], rb2[:, :cw])
                nc.vector.reciprocal_approx_fast(rb1[:, :cw], rb1[:, :cw])
                t1 = pep.tile([65, QC], f32, tag="t1")
                nc.vector.tensor_mul(t1[:, :cw], sa1[:, sl], rb1[:, :cw])
                t2 = pep.tile([65, QC], f32, tag="t2")
                nc.vector.tensor_mul(t2[:, :cw], sa2[:, sl], rb2[:, :cw])
                # outc = t1 - lam * t2  (row 0: lam-col is 1.0 -> exact 0)
                nc.vector.ln_bwd_dx(
                    outc[:, sl],
                    dy=t1[:, :cw],
                    x_hat=t2[:, :cw],
                    mean_dyx=gbs[:, 2:3],
                    mean_dy=0.0,
                    scale=1.0,
                )
                nc.vector.bn_stats(st[:, ci, :], outc[:, sl])

            def emit_pv(u):
                ci, k, half = u_ci[u], u_k[u], u_half[u]
                cw = cws[ci]
                if acc[ci] is None:
                    acc[ci] = (
                        pacc.tile([65, QC], f32, tag="a1", name="a1"),
                        pacc.tile([65, QC], f32, tag="a2", name="a2"),
                    )
                e = scs[u_tile[u]][1]
                nc.tensor.matmul(
                    acc[ci][half][:, :cw],
                    lhsT=vs[:, ts(k, 65)],
                    rhs=e[:, u_off[u] : u_off[u] + cw],
                    start=(k == 0),
                    stop=(k == KB - 1),
                )
                if k == KB - 1 and half == 1:
                    emit_epilogue(ci)
                    acc[ci] = None

            def flush_pvs():
                # Emit PVs for pending units. Normal lag: two acts beyond
                # the unit's own tile, so in the in-order PE queue the
                # NEXT tile's QKs precede these PVs (which block on the
                # previous act's exp + 100ns sem propagation) — the
                # following act's QK dependency then resolves well before
                # the act engine is free, instead of ~150ns late.
                # Chunk-first units: one act further, so the previous
                # chunk's accumulator eviction can drain.
                while pend:
                    u = pend[0]
                    req = u_tile[u] + 3 + (1 if u_k[u] == 0 else 0)
                    if n_acts < req:
                        break
                    pend.pop(0)
                    emit_pv(u)

            next_act = 0
            for u in range(UH):
                ci, k, half = u_ci[u], u_k[u], u_half[u]
                t = u_tile[u]
                cw = cws[ci]
                if u_off[u] == 0:
                    scs[t] = (
                        psc.tile([128, 3 * QC], f32, tag="sc", name="sc_t"),
                        pe.tile([128, 3 * QC], f16, name="e_t"),
                    )
                ksk = ksh[k // 8][:, ts(k % 8, 128)]
                cs0 = css[ci]
                qt = qsh[cs0 // QC]
                qo = cs0 % QC
                nc.tensor.matmul(
                    scs[t][0][:, u_off[u] : u_off[u] + cw],
                    lhsT=ksk[64 * half : 64 * half + 64, :],
                    rhs=qt[64 * half : 64 * half + 64, qo : qo + cw],
                    start=True,
                    stop=True,
                )
                # after three full QK pairs (acts t0/t1 covered), emit the
                # previous head's deferred tail
                if half == 1 and u == 5 and prev_tail[0] is not None:
                    prev_tail[0]()
                    prev_tail[0] = None
                # prefetch the next head's tensors mid-stream, clear of
                # both this head's loads and its finalize out-DMAs; only
                # between QK pairs so the pair stays PE-adjacent
                if half == 1 and u == UH // 2 + 1 and h + 1 < HPC:
                    nxt_loads = emit_loads(h + 1)
                if half == 1:
                    while next_act < NT and tiles[next_act][1] <= u:
                        lo, hi, w = tiles[next_act]
                        sc, e = scs[next_act]
                        nc.scalar.activation(
                            e[:, 0:w], sc[:, 0:w], AF.Exp, scale=SCALE
                        )
                        n_acts += 1
                        pend.extend(range(lo, hi + 1))
                        next_act += 1
                        flush_pvs()
            flush_pvs()

            def drain_and_finalize():
                while pend:
                    emit_pv(pend.pop(0))

                # ---- head finalize (partition 0 rows: harmless zeros) ----
                mv = pst.tile([65, 2], f32)
                nc.vector.bn_aggr(mv[:], st[:, :NCH, :])
                s2 = pst.tile([65, 2], f32)
                nc.vector.tensor_copy(s2[:, 0:1], mv[:, 0:1])
                # E[x^2]_p = var_p + mean_p^2
                nc.vector.tensor_scalar(
                    out=s2[:, 1:2],
                    in0=mv[:, 0:1],
                    scalar1=mv[:, 0:1],
                    scalar2=mv[:, 1:2],
                    op0=OP.mult,
                    op1=OP.add,
                )
                tot = pst.tile([65, 2], f32)
                nc.gpsimd.partition_all_reduce(
                    tot[:], s2[:], channels=65, reduce_op=bass_isa.ReduceOp.add
                )
                # tot = sums over partitions of per-partition (mean, E[x^2])
                # over 2048 elements; rows 1-64 carry signal -> /64.
                mu = pst.tile([65, 1], f32)
                nc.vector.tensor_scalar_mul(mu[:], tot[:, 0:1], 1.0 / 64.0)
                # veps = (tot1 - tot0*mu - (-64*eps))/64 = var + eps, fused
                # into one ln_bwd_dx pass (tot0*mu = 64*mu^2).
                veps = pst.tile([65, 1], f32)
                nc.vector.ln_bwd_dx(
                    veps[:],
                    dy=tot[:, 1:2],
                    x_hat=tot[:, 0:1],
                    mean_dyx=mu[:],
                    mean_dy=-64.0 * EPS,
                    scale=1.0 / 64.0,
                )
                if last_h:
                    # rstd = sqrt(1/veps): fast DVE reciprocal + ScalarE
                    # sqrt (table pre-loaded right after the last exp act;
                    # ScalarE is idle in the tail).
                    rv = pst.tile([65, 1], f32)
                    nc.vector.reciprocal_approx_fast(rv[:], veps[:])
                    rstd = pst.tile([65, 1], f32)
                    nc.scalar.activation(rstd[:], rv[:], AF.Sqrt)
                    cur = rstd[:]
                else:
                    # rstd = Quake-rsqrt on DVE (bitcast + Newton) so
                    # ScalarE stays on the exp table mid-stream.
                    ish = pst.tile([65, 1], i32)
                    nc.vector.tensor_scalar(
                        out=ish[:],
                        in0=veps[:].bitcast(i32),
                        scalar1=1,
                        scalar2=None,
                        op0=OP.logical_shift_right,
                    )
                    iy = pst.tile([65, 1], i32)
                    nc.vector.tensor_sub(iy[:], magic[:], ish[:])
                    vh = pst.tile([65, 1], f32)
                    nc.vector.tensor_scalar_mul(vh[:], veps[:], -0.5)
                    cur = iy[:].bitcast(f32)
                    for it in range(2):
                        aa = pst.tile([65, 1], f32, tag=f"nr_a{it}")
                        nc.vector.tensor_mul(aa[:], cur, cur)
                        bb = pst.tile([65, 1], f32, tag=f"nr_b{it}")
                        nc.vector.tensor_scalar(
                            out=bb[:], in0=aa[:], scalar1=vh[:], scalar2=1.5,
                            op0=OP.mult, op1=OP.add,
                        )
                        nxt = pst.tile([65, 1], f32, tag=f"nr_y{it}")
                        nc.vector.tensor_tensor(
                            out=nxt[:], in0=bb[:], in1=cur, op=OP.mult
                        )
                        cur = nxt[:]
                sg = pst.tile([65, 1], f32)
                nc.vector.tensor_tensor(
                    out=sg[:], in0=cur, in1=gbs[:, 0:1], op=OP.mult
                )
                tb = pst.tile([65, 1], f32)
                ms = pst.tile([65, 1], f32)
                nc.vector.tensor_scalar(
                    out=ms[:], in0=mu[:], scalar1=sg[:], scalar2=None, op0=OP.mult
                )
                nc.vector.tensor_sub(tb[:], gbs[:, 1:2], ms[:])
                # final affine, in pieces so each piece's output DMA
                # overlaps the next piece's apply. In the exposed tail
                # (last head) pieces alternate ScalarE/DVE and the DMAs
                # rotate over three queues (scalar's DMA follows its own
                # affine piece on the same queue); mid-stream heads stay
                # off ScalarE.
                n_pieces = 4 if last_h else 2
                outf = pout.tile([65, S], f32)
                dmaq = [nc.sync, nc.gpsimd, nc.scalar, nc.gpsimd]
                for piece in range(n_pieces):
                    sl = slice(
                        piece * (S // n_pieces), (piece + 1) * (S // n_pieces)
                    )
                    if last_h and piece % 2 == 0:
                        nc.scalar.activation(
                            outf[:, sl], outc[:, sl], AF.Identity,
                            bias=tb[:], scale=sg[:],
                        )
                    else:
                        nc.vector.tensor_scalar(
                            out=outf[:, sl],
                            in0=outc[:, sl],
                            scalar1=sg[:],
                            scalar2=tb[:],
                            op0=OP.mult,
                            op1=OP.add,
                        )
                    if last_h:
                        dmaq[piece].dma_start(outT[h, :, sl], outf[1:65, sl])
                    else:
                        nc.sync.dma_start(outT[h, :, sl], outf[1:65, sl])

            prev_tail[0] = drain_and_finalize
            return nxt_loads

        lds = emit_loads(0, startup=True)
        for h in range(HPC):
            lds = run_head(h, lds)
        prev_tail[0]()

    nc.compile()
    return nc


def _get_nc():
    if "nc" not in _CACHE:
        _CACHE["nc"] = _build_nc()
    return _CACHE["nc"]


def _host_prep(q, k, v, lq1, lq2, lk1, lk2, gamma, beta):
    """Build per-core input maps."""
    q = np.asarray(q, dtype=np.float32)
    k = np.asarray(k, dtype=np.float32)
    v = np.asarray(v, dtype=np.float32)
    lam = float(
        np.exp(np.float32(np.dot(lq1, lk1)))
        - np.exp(np.float32(np.dot(lq2, lk2)))
        + LAMBDA_INIT
    )
    g2 = (np.asarray(gamma, np.float32) * (1.0 - LAMBDA_INIT)).reshape(H, D)
    b2 = (np.asarray(beta, np.float32) * (1.0 - LAMBDA_INIT)).reshape(H, D)

    in_maps = []
    for c in range(N_CORES):
        heads = range(c * HPC, (c + 1) * HPC)
        qTa = np.empty((HPC, 128, S), np.float16)
        kTa = np.empty((HPC, 128, S), np.float16)
        vpa = np.empty((HPC, 128, KB * 65), np.float16)
        gba = np.empty((HPC, 64, 3), np.float32)
        for i, hh in enumerate(heads):
            qTa[i] = q[0, hh].T.astype(np.float16)
            kTa[i] = k[0, hh].T.astype(np.float16)
            vh = v[0, hh]  # [S, 64]
            v1 = np.concatenate([np.ones((S, 1), np.float32), vh], axis=1)
            # SBUF image: [partition(key within block), kblock*65 + col]
            vpa[i] = (
                v1.reshape(KB, 128, 65).transpose(1, 0, 2).reshape(128, KB * 65)
            ).astype(np.float16)
            gba[i, :, 0] = g2[hh]
            gba[i, :, 1] = b2[hh]
            gba[i, :, 2] = lam
        in_maps.append({"qT": qTa, "kT": kTa, "vp": vpa, "gb": gba})
    return in_maps


def kernel(q, k, v, lq1, lq2, lk1, lk2, gamma, beta, _trace=False, _tmpdir=None):
    from concourse.bass_utils import run_bass_kernel_spmd

    nc = _get_nc()
    in_maps = _host_prep(q, k, v, lq1, lq2, lk1, lk2, gamma, beta)
    res = run_bass_kernel_spmd(
        nc,
        in_maps,
        core_ids=list(range(N_CORES)),
        trace=_trace,
        tmpdir=_tmpdir,
    )
    out = np.empty((B, H, S, D), np.float32)
    for c in range(N_CORES):
        outT = res.results[c]["outT"]  # [HPC, 64, S]
        for i in range(HPC):
            out[0, c * HPC + i] = outT[i].T
    if _trace:
        _CACHE["last_results"] = res
    return out



# revision 15
# speedup vs baseline: 1.1104x; 1.0085x over previous
"""Differential attention (two-softmax diff + GroupNorm) on 8 TRN2 cores.

Sharding: 16 heads / 8 cores = 2 heads per core (head-parallel, no
collectives). GroupNorm stats are per-(batch, head) so each core is fully
independent.

Device layout choices (host prepares everything):
  - Q, K per head are host-transposed to [128(d), 2048(s)] fp16: partitions
    0-63 hold half-1 (q1/k1), partitions 64-127 hold half-2. QK^T then
    contracts over the partition dim directly, producing transposed score
    blocks S^T[key, query] in PSUM (fp32). The two 64-contraction QK
    matmuls issue back-to-back and the PE runs them concurrently on row
    groups h0/h64.
  - V per head is prefixed with a ones column (V' = [1 | V], 65 cols, fp16)
    and pre-arranged into the SBUF image [128(key of block), 16*65]: the PV
    matmul (lhsT = V'[kblk], rhs = exp(S^T)[kblk]) then yields the softmax
    denominator on partition 0 and the numerator on partitions 1-64 in one
    accumulation group. BOTH halves share the same V' stationary (w1*V and
    lam*w2*V use the same V); lam is applied in the per-chunk epilogue via
    the fused LN_BWD_DX DVE op (out = t1 - lam*t2).
  - Output stays in [d, q] layout on device; the host transposes it back.

fp16 is used on every matmul path: it streams through the PE at ~1
cycle/column with the same 10-bit mantissa class as tf32. exp() runs on
ScalarE straight out of PSUM, writing fp16.

Main loop per (head, 512-query chunk): 16 key blocks of
  QK matmul pair -> exp on ScalarE (PSUM -> SBUF) -> PV pair accumulate,
then a per-chunk epilogue slice (denominator broadcast on GpSimd, divide +
combine + bn_stats on DVE) that hides under later chunks' main loop.
ScalarE's exp stream is the pacing engine (~1.1us per key block); the PE
fits just beneath it, so no warm-up spinner is used (the HAM clock gate
settles by itself and junk matmuls only delay the first real block).

Tail: rstd = sqrt(reciprocal_approx_fast(var+eps)) with the Sqrt act
table pre-loaded via a dummy activation right after the last exp, and the
final affine + output DMA run in 4 interleaved pieces.
"""

import math

import numpy as np

B, H, S, D = 1, 16, 2048, 64
N_CORES = 8
HPC = H // N_CORES  # heads per core
QC = 512            # query-chunk width (PSUM bank budget)
N_QC = S // QC
KB = S // 128       # key blocks of 128
LAMBDA_INIT = 0.8
EPS = 1e-5
SCALE = 1.0 / math.sqrt(D)
N_WARMUP_MM = 10

_CACHE = {}


def _build_nc():
    from contextlib import ExitStack

    import concourse.bacc as bacc
    import concourse.bass as bass
    import concourse.tile as tile
    from concourse import bass_isa, mybir

    f32 = mybir.dt.float32
    f16 = mybir.dt.float16
    i32 = mybir.dt.int32
    AF = mybir.ActivationFunctionType
    OP = mybir.AluOpType
    ts = bass.ts

    nc = bacc.Bacc("TRN2", target_bir_lowering=False, debug=False)

    qT = nc.dram_tensor("qT", [HPC, 128, S], f16, kind="ExternalInput").ap()
    kT = nc.dram_tensor("kT", [HPC, 128, S], f16, kind="ExternalInput").ap()
    vp = nc.dram_tensor("vp", [HPC, 128, KB * 65], f16, kind="ExternalInput").ap()
    # per-head (gamma', beta', lam) columns; row 0 of the lam column is 1.0
    gb = nc.dram_tensor("gb", [HPC, 64, 3], f32, kind="ExternalInput").ap()
    outT = nc.dram_tensor("outT", [HPC, 64, S], f32, kind="ExternalOutput").ap()

    with tile.TileContext(nc) as tc, ExitStack() as ctx:
        pq = ctx.enter_context(tc.tile_pool(name="pq", bufs=2))
        pk = ctx.enter_context(tc.tile_pool(name="pk", bufs=2))
        pv = ctx.enter_context(tc.tile_pool(name="pv", bufs=2))
        pe = ctx.enter_context(tc.tile_pool(name="pe", bufs=4))
        psa = ctx.enter_context(tc.tile_pool(name="psa", bufs=2))
        pep = ctx.enter_context(tc.tile_pool(name="pep", bufs=2))
        pout = ctx.enter_context(tc.tile_pool(name="pout", bufs=2))
        pst = ctx.enter_context(tc.tile_pool(name="pst", bufs=2))
        psingle = ctx.enter_context(tc.tile_pool(name="psingle", bufs=1))
        psc = ctx.enter_context(tc.tile_pool(name="psc", bufs=2, space="PSUM"))
        pacc = ctx.enter_context(tc.tile_pool(name="pacc", bufs=1, space="PSUM"))

        eps_t = psingle.tile([65, 1], f32)
        nc.vector.memset(eps_t, EPS)
        ones65 = psingle.tile([1, 65], f32)
        nc.vector.memset(ones65, 1.0)
        junk = psingle.tile([65, 1], f32)
        nc.vector.memset(junk, 1.0)
        magic = psingle.tile([65, 1], i32)
        nc.vector.memset(magic, 0x5F3759DF)

        # PE warm-up: ~24 tiny back-to-back matmuls flip the HAM clock
        # gate toward 8/8 while the first head's DMAs are in flight. The
        # warm-up accumulator borrows the a1 slot; the first chunk's a1
        # allocation simply waits for the last warm-up matmul.
        wu_w = psingle.tile([128, 128], f16)
        nc.vector.memset(wu_w, 0.0)
        wu_ps = pacc.tile([128, 128], f32, tag="a1")
        for _ in range(N_WARMUP_MM):
            nc.tensor.matmul(
                wu_ps[:], lhsT=wu_w[:], rhs=wu_w[:], start=True, stop=True
            )

        def emit_loads(h, startup=False):
            """DMA in head h's tensors. At startup the loads spread over
            three queues (sync: K, gpsimd: Q + gb, vector: V') so the
            transfers run in parallel and the first key blocks' data
            (K[0:256], Q chunk 0) leads each queue; the mid-stream
            prefetch for the next head stays on the Sync queue (it is
            fully hidden under the exp stream)."""
            ksh = [pk.tile([128, S // 2], f16, tag=f"ks{j}", name="ks") for j in range(2)]
            qsh = [pq.tile([128, QC], f16, tag=f"qs{j}", name="qs") for j in range(N_QC)]
            vs = pv.tile([128, KB * 65], f16, tag="v")
            vw = KB * 65 // 4
            if startup:
                # Per-queue transfer rate is only ~34GB/s (1KB lines), so
                # the plan splits the critical first tensors (K[0:256] +
                # q0) three ways and then pipelines each queue in
                # consumption order (the gpsimd queue frees ~1us before
                # sync/scalar, so it leads with q0's left half):
                gbs = pst.tile([65, 3], f32, tag="gbs")
                nc.vector.memset(gbs[0:1, :], 0.0)
                nc.vector.memset(gbs[0:1, 2:3], 1.0)
                nc.gpsimd.dma_start(qsh[0][:, 0:256], qT[h, :, 0:256])
                nc.gpsimd.dma_start(gbs[1:65, :], gb[h])
                nc.scalar.dma_start(ksh[0][:, 0:256], kT[h, :, 0:256])
                nc.sync.dma_start(qsh[0][:, 256:QC], qT[h, :, 256:QC])
                nc.gpsimd.dma_start(vs[:, 0:vw], vp[h, :, 0:vw])
                nc.sync.dma_start(ksh[0][:, 256:512], kT[h, :, 256:512])
                nc.gpsimd.dma_start(ksh[1][:, 0:512], kT[h, :, 1024:1536])
                nc.sync.dma_start(ksh[0][:, 512:768], kT[h, :, 512:768])
                nc.gpsimd.dma_start(vs[:, vw : 2 * vw], vp[h, :, vw : 2 * vw])
                nc.sync.dma_start(ksh[0][:, 768:1024], kT[h, :, 768:1024])
                nc.gpsimd.dma_start(qsh[1][:], qT[h, :, QC : 2 * QC])
                nc.sync.dma_start(ksh[1][:, 512:1024], kT[h, :, 1536:2048])
                nc.gpsimd.dma_start(vs[:, 2 * vw : 3 * vw], vp[h, :, 2 * vw : 3 * vw])
                nc.gpsimd.dma_start(vs[:, 3 * vw :], vp[h, :, 3 * vw :])
                nc.gpsimd.dma_start(qsh[2][:], qT[h, :, 2 * QC : 3 * QC])
                nc.gpsimd.dma_start(qsh[3][:], qT[h, :, 3 * QC : 4 * QC])
                return ksh, qsh, vs, gbs
            else:
                nc.sync.dma_start(ksh[0][:, 0:256], kT[h, :, 0:256])
                nc.sync.dma_start(qsh[0][:], qT[h, :, 0:QC])
                nc.sync.dma_start(ksh[0][:, 256 : S // 2], kT[h, :, 256 : S // 2])
                nc.sync.dma_start(qsh[1][:], qT[h, :, QC : 2 * QC])
                nc.sync.dma_start(vs[:, 0:vw], vp[h, :, 0:vw])
                nc.sync.dma_start(vs[:, vw : 2 * vw], vp[h, :, vw : 2 * vw])
                nc.sync.dma_start(ksh[1][:], kT[h, :, S // 2 : S])
                nc.sync.dma_start(vs[:, 2 * vw : 3 * vw], vp[h, :, 2 * vw : 3 * vw])
                nc.sync.dma_start(vs[:, 3 * vw :], vp[h, :, 3 * vw :])
                for j in range(2, N_QC):
                    nc.sync.dma_start(qsh[j][:], qT[h, :, j * QC : (j + 1) * QC])
            gbs = pst.tile([65, 3], f32, tag="gbs")
            nc.vector.memset(gbs[0:1, :], 0.0)
            nc.vector.memset(gbs[0:1, 2:3], 1.0)
            nc.gpsimd.dma_start(gbs[1:65, :], gb[h])
            return ksh, qsh, vs, gbs

        # Deferred per-head tail: the previous head's last-tile PVs,
        # epilogue and finalize are emitted a few QK pairs into the NEXT
        # head's stream, so they don't sit between the last act and the
        # next head's first QKs in the in-order PE queue (same fix as the
        # chunk-boundary PV deferral, applied at the head seam).
        prev_tail = [None]

        def run_head(h, loads):
            ksh, qsh, vs, gbs = loads
            nxt_loads = None
            last_h = h == HPC - 1

            # Query-chunk layout. The last head tapers to two 256-wide
            # chunks at the end so the final (exposed) epilogue's DVE
            # chain is half length; mid-stream epilogues hide under the
            # exp stream either way.
            cws = [512, 512, 512, 256, 256] if last_h else [QC] * N_QC
            NCH = len(cws)
            css = [sum(cws[:i]) for i in range(NCH)]

            # Units: one (chunk, key-block, half) score block of cw
            # columns; chunks outer, then k, then half.
            u_ci, u_k, u_half = [], [], []
            for ci in range(NCH):
                for k in range(KB):
                    for half in (0, 1):
                        u_ci.append(ci)
                        u_k.append(k)
                        u_half.append(half)
            UH = len(u_ci)

            # Tiles: pack units into <=1536 score columns (3 PSUM banks);
            # each exp act covers one tile. 512-wide units go 3 per tile
            # at natural offsets. 256-wide units go 6 per tile with
            # PERMUTED offsets so the column-bank sequence is 0,1,2,0,1,2:
            # the QK pair (h0/h64 row groups) runs concurrently on the PE
            # and two in-flight matmuls draining into the same PSUM bank
            # is a fatal collision — adjacent units must differ in bank.
            u_tile, u_off, tiles = [], [], []
            i = 0
            while i < UH:
                if cws[u_ci[i]] == 512:
                    j = i
                    while j < UH and j - i < 3 and cws[u_ci[j]] == 512:
                        j += 1
                    offs = [512 * t for t in range(j - i)]
                    w = 512 * (j - i)
                else:
                    j = i
                    while j < UH and j - i < 6 and cws[u_ci[j]] == 256:
                        j += 1
                    n = j - i  # always even (half pairs)
                    if n == 6:
                        offs = [0, 512, 1024, 256, 768, 1280]
                        w = 1536
                    else:
                        offs = [0, 512, 256, 768][:n]
                        w = 1024 if n == 4 else 768
                for t, u in enumerate(range(i, j)):
                    u_tile.append(len(tiles))
                    u_off.append(offs[t])
                tiles.append((i, j - 1, w))
                i = j
            NT = len(tiles)

            # [denominator(row 0) | numerator(rows 1-64)] x all queries
            sa1 = psa.tile([65, S], f32)
            sa2 = psa.tile([65, S], f32)
            outc = pout.tile([65, S], f32)
            st = pst.tile([65, 5, 6], f32, tag="st")

            scs = {}
            acc = [None] * NCH
            pend = []  # units whose act is emitted but PV is not
            n_acts = 0

            def emit_epilogue(ci):
                a1, a2 = acc[ci]
                cs0, cw = css[ci], cws[ci]
                sl = slice(cs0, cs0 + cw)
                last = last_h and ci == NCH - 1
                # evict accumulators to SBUF; the next chunk's first PVs
                # are deferred one extra act so this drain can finish.
                nc.vector.tensor_copy(sa1[:, sl], a1[:, :cw])
                if last:
                    nc.scalar.copy(sa2[:, sl], a2[:, :cw])
                    # pre-load the Sqrt act table while the epilogue runs.
                    # The dummy must DEPEND on tail data (the sa2 eviction):
                    # with only an early dep it bypasses the queued exp acts
                    # via the wait-queue and runs at startup, so the real
                    # Sqrt pays a fresh table load on the critical chain.
                    nc.scalar.activation(
                        junk[:], sa2[:, cs0 : cs0 + 1], AF.Sqrt
                    )
                else:
                    nc.vector.tensor_copy(sa2[:, sl], a2[:, :cw])

                rb1 = pep.tile([65, QC], f32, tag="rb1")
                nc.gpsimd.partition_broadcast(
                    rb1[:, :cw], sa1[0:1, sl], channels=65
                )
                rb2 = pep.tile([65, QC], f32, tag="rb2")
                if last:
                    rb2_ps = pacc.tile([65, QC], f32, tag="a1")
                    nc.tensor.matmul(
                        rb2_ps[:, :cw],
                        lhsT=ones65[:],
                        rhs=sa2[0:1, sl],
                        start=True,
                        stop=True,
                    )
                    nc.vector.reciprocal_approx_fast(rb2[:, :cw], rb2_ps[:, :cw])
                else:
                    nc.gpsimd.partition_broadcast(
                        rb2[:, :cw], sa2[0:1, sl], channels=65
                    )
                    nc.vector.reciprocal_approx_fast(rb2[:, :cw], rb2[:, :cw])
                nc.vector.reciprocal_approx_fast(rb1[:, :cw], rb1[:, :cw])
                t1 = pep.tile([65, QC], f32, tag="t1")
                nc.vector.tensor_mul(t1[:, :cw], sa1[:, sl], rb1[:, :cw])
                t2 = pep.tile([65, QC], f32, tag="t2")
                nc.vector.tensor_mul(t2[:, :cw], sa2[:, sl], rb2[:, :cw])
                # outc = t1 - lam * t2  (row 0: lam-col is 1.0 -> exact 0)
                nc.vector.ln_bwd_dx(
                    outc[:, sl],
                    dy=t1[:, :cw],
                    x_hat=t2[:, :cw],
                    mean_dyx=gbs[:, 2:3],
                    mean_dy=0.0,
                    scale=1.0,
                )
                nc.vector.bn_stats(st[:, ci, :], outc[:, sl])

            def emit_pv(u):
                ci, k, half = u_ci[u], u_k[u], u_half[u]
                cw = cws[ci]
                if acc[ci] is None:
                    acc[ci] = (
                        pacc.tile([65, QC], f32, tag="a1", name="a1"),
                        pacc.tile([65, QC], f32, tag="a2", name="a2"),
                    )
                e = scs[u_tile[u]][1]
                nc.tensor.matmul(
                    acc[ci][half][:, :cw],
                    lhsT=vs[:, ts(k, 65)],
                    rhs=e[:, u_off[u] : u_off[u] + cw],
                    start=(k == 0),
                    stop=(k == KB - 1),
                )
                if k == KB - 1 and half == 1:
                    emit_epilogue(ci)
                    acc[ci] = None

            def flush_pvs():
                # Emit PVs for pending units. Normal lag: two acts beyond
                # the unit's own tile, so in the in-order PE queue the
                # NEXT tile's QKs precede these PVs (which block on the
                # previous act's exp + 100ns sem propagation) — the
                # following act's QK dependency then resolves well before
                # the act engine is free, instead of ~150ns late.
                # Chunk-first units: one act further, so the previous
                # chunk's accumulator eviction can drain.
                while pend:
                    u = pend[0]
                    req = u_tile[u] + 3 + (1 if u_k[u] == 0 else 0)
                    if n_acts < req:
                        break
                    pend.pop(0)
                    emit_pv(u)

            next_act = 0
            for u in range(UH):
                ci, k, half = u_ci[u], u_k[u], u_half[u]
                t = u_tile[u]
                cw = cws[ci]
                if u_off[u] == 0:
                    scs[t] = (
                        psc.tile([128, 3 * QC], f32, tag="sc", name="sc_t"),
                        pe.tile([128, 3 * QC], f16, name="e_t"),
                    )
                ksk = ksh[k // 8][:, ts(k % 8, 128)]
                cs0 = css[ci]
                qt = qsh[cs0 // QC]
                qo = cs0 % QC
                nc.tensor.matmul(
                    scs[t][0][:, u_off[u] : u_off[u] + cw],
                    lhsT=ksk[64 * half : 64 * half + 64, :],
                    rhs=qt[64 * half : 64 * half + 64, qo : qo + cw],
                    start=True,
                    stop=True,
                )
                # after three full QK pairs (acts t0/t1 covered), emit the
                # previous head's deferred tail
                if half == 1 and u == 5 and prev_tail[0] is not None:
                    prev_tail[0]()
                    prev_tail[0] = None
                # prefetch the next head's tensors mid-stream, clear of
                # both this head's loads and its finalize out-DMAs; only
                # between QK pairs so the pair stays PE-adjacent
                if half == 1 and u == UH // 2 + 1 and h + 1 < HPC:
                    nxt_loads = emit_loads(h + 1)
                if half == 1:
                    while next_act < NT and tiles[next_act][1] <= u:
                        lo, hi, w = tiles[next_act]
                        sc, e = scs[next_act]
                        nc.scalar.activation(
                            e[:, 0:w], sc[:, 0:w], AF.Exp, scale=SCALE
                        )
                        n_acts += 1
                        pend.extend(range(lo, hi + 1))
                        next_act += 1
                        flush_pvs()
            flush_pvs()

            def drain_and_finalize():
                while pend:
                    emit_pv(pend.pop(0))

                # ---- head finalize (partition 0 rows: harmless zeros) ----
                mv = pst.tile([65, 2], f32)
                nc.vector.bn_aggr(mv[:], st[:, :NCH, :])
                s2 = pst.tile([65, 2], f32)
                nc.vector.tensor_copy(s2[:, 0:1], mv[:, 0:1])
                # E[x^2]_p = var_p + mean_p^2
                nc.vector.tensor_scalar(
                    out=s2[:, 1:2],
                    in0=mv[:, 0:1],
                    scalar1=mv[:, 0:1],
                    scalar2=mv[:, 1:2],
                    op0=OP.mult,
                    op1=OP.add,
                )
                tot = pst.tile([65, 2], f32)
                nc.gpsimd.partition_all_reduce(
                    tot[:], s2[:], channels=65, reduce_op=bass_isa.ReduceOp.add
                )
                # tot = sums over partitions of per-partition (mean, E[x^2])
                # over 2048 elements; rows 1-64 carry signal -> /64.
                mu = pst.tile([65, 1], f32)
                nc.vector.tensor_scalar_mul(mu[:], tot[:, 0:1], 1.0 / 64.0)
                # veps = (tot1 - tot0*mu - (-64*eps))/64 = var + eps, fused
                # into one ln_bwd_dx pass (tot0*mu = 64*mu^2).
                veps = pst.tile([65, 1], f32)
                nc.vector.ln_bwd_dx(
                    veps[:],
                    dy=tot[:, 1:2],
                    x_hat=tot[:, 0:1],
                    mean_dyx=mu[:],
                    mean_dy=-64.0 * EPS,
                    scale=1.0 / 64.0,
                )
                if last_h:
                    # rstd = sqrt(1/veps): fast DVE reciprocal + ScalarE
                    # sqrt (table pre-loaded right after the last exp act;
                    # ScalarE is idle in the tail).
                    rv = pst.tile([65, 1], f32)
                    nc.vector.reciprocal_approx_fast(rv[:], veps[:])
                    rstd = pst.tile([65, 1], f32)
                    nc.scalar.activation(rstd[:], rv[:], AF.Sqrt)
                    cur = rstd[:]
                else:
                    # rstd = Quake-rsqrt on DVE (bitcast + Newton) so
                    # ScalarE stays on the exp table mid-stream.
                    ish = pst.tile([65, 1], i32)
                    nc.vector.tensor_scalar(
                        out=ish[:],
                        in0=veps[:].bitcast(i32),
                        scalar1=1,
                        scalar2=None,
                        op0=OP.logical_shift_right,
                    )
                    iy = pst.tile([65, 1], i32)
                    nc.vector.tensor_sub(iy[:], magic[:], ish[:])
                    vh = pst.tile([65, 1], f32)
                    nc.vector.tensor_scalar_mul(vh[:], veps[:], -0.5)
                    cur = iy[:].bitcast(f32)
                    for it in range(2):
                        aa = pst.tile([65, 1], f32, tag=f"nr_a{it}")
                        nc.vector.tensor_mul(aa[:], cur, cur)
                        bb = pst.tile([65, 1], f32, tag=f"nr_b{it}")
                        nc.vector.tensor_scalar(
                            out=bb[:], in0=aa[:], scalar1=vh[:], scalar2=1.5,
                            op0=OP.mult, op1=OP.add,
                        )
                        nxt = pst.tile([65, 1], f32, tag=f"nr_y{it}")
                        nc.vector.tensor_tensor(
                            out=nxt[:], in0=bb[:], in1=cur, op=OP.mult
                        )
                        cur = nxt[:]
                sg = pst.tile([65, 1], f32)
                nc.vector.tensor_tensor(
                    out=sg[:], in0=cur, in1=gbs[:, 0:1], op=OP.mult
                )
                tb = pst.tile([65, 1], f32)
                ms = pst.tile([65, 1], f32)
                nc.vector.tensor_scalar(
                    out=ms[:], in0=mu[:], scalar1=sg[:], scalar2=None, op0=OP.mult
                )
                nc.vector.tensor_sub(tb[:], gbs[:, 1:2], ms[:])
                # final affine, in pieces so each piece's output DMA
                # overlaps the next piece's apply. In the exposed tail
                # (last head) pieces alternate ScalarE/DVE and the DMAs
                # rotate over three queues (scalar's DMA follows its own
                # affine piece on the same queue); mid-stream heads stay
                # off ScalarE.
                # keep the gpsimd queue OUT of the final out-DMAs: its
                # end-of-program drain has a ~2us fixed quiesce cost that
                # then overlaps the sync/scalar DMA completions instead of
                # serializing after them.
                n_pieces = 4 if last_h else 2
                outf = pout.tile([65, S], f32)
                dmaq = [nc.sync, nc.sync, nc.scalar, nc.scalar]
                for piece in range(n_pieces):
                    sl = slice(
                        piece * (S // n_pieces), (piece + 1) * (S // n_pieces)
                    )
                    if last_h and piece % 2 == 0:
                        nc.scalar.activation(
                            outf[:, sl], outc[:, sl], AF.Identity,
                            bias=tb[:], scale=sg[:],
                        )
                    else:
                        nc.vector.tensor_scalar(
                            out=outf[:, sl],
                            in0=outc[:, sl],
                            scalar1=sg[:],
                            scalar2=tb[:],
                            op0=OP.mult,
                            op1=OP.add,
                        )
                    if last_h:
                        dmaq[piece].dma_start(outT[h, :, sl], outf[1:65, sl])
                    else:
                        nc.sync.dma_start(outT[h, :, sl], outf[1:65, sl])

            prev_tail[0] = drain_and_finalize
            return nxt_loads

        lds = emit_loads(0, startup=True)
        for h in range(HPC):
            lds = run_head(h, lds)
        prev_tail[0]()

    nc.compile()
    return nc


def _get_nc():
    if "nc" not in _CACHE:
        _CACHE["nc"] = _build_nc()
    return _CACHE["nc"]


def _host_prep(q, k, v, lq1, lq2, lk1, lk2, gamma, beta):
    """Build per-core input maps."""
    q = np.asarray(q, dtype=np.float32)
    k = np.asarray(k, dtype=np.float32)
    v = np.asarray(v, dtype=np.float32)
    lam = float(
        np.exp(np.float32(np.dot(lq1, lk1)))
        - np.exp(np.float32(np.dot(lq2, lk2)))
        + LAMBDA_INIT
    )
    g2 = (np.asarray(gamma, np.float32) * (1.0 - LAMBDA_INIT)).reshape(H, D)
    b2 = (np.asarray(beta, np.float32) * (1.0 - LAMBDA_INIT)).reshape(H, D)

    in_maps = []
    for c in range(N_CORES):
        heads = range(c * HPC, (c + 1) * HPC)
        qTa = np.empty((HPC, 128, S), np.float16)
        kTa = np.empty((HPC, 128, S), np.float16)
        vpa = np.empty((HPC, 128, KB * 65), np.float16)
        gba = np.empty((HPC, 64, 3), np.float32)
        for i, hh in enumerate(heads):
            qTa[i] = q[0, hh].T.astype(np.float16)
            kTa[i] = k[0, hh].T.astype(np.float16)
            vh = v[0, hh]  # [S, 64]
            v1 = np.concatenate([np.ones((S, 1), np.float32), vh], axis=1)
            # SBUF image: [partition(key within block), kblock*65 + col]
            vpa[i] = (
                v1.reshape(KB, 128, 65).transpose(1, 0, 2).reshape(128, KB * 65)
            ).astype(np.float16)
            gba[i, :, 0] = g2[hh]
            gba[i, :, 1] = b2[hh]
            gba[i, :, 2] = lam
        in_maps.append({"qT": qTa, "kT": kTa, "vp": vpa, "gb": gba})
    return in_maps


def kernel(q, k, v, lq1, lq2, lk1, lk2, gamma, beta, _trace=False, _tmpdir=None):
    from concourse.bass_utils import run_bass_kernel_spmd

    nc = _get_nc()
    in_maps = _host_prep(q, k, v, lq1, lq2, lk1, lk2, gamma, beta)
    res = run_bass_kernel_spmd(
        nc,
        in_maps,
        core_ids=list(range(N_CORES)),
        trace=_trace,
        tmpdir=_tmpdir,
    )
    out = np.empty((B, H, S, D), np.float32)
    for c in range(N_CORES):
        outT = res.results[c]["outT"]  # [HPC, 64, S]
        for i in range(HPC):
            out[0, c * HPC + i] = outT[i].T
    if _trace:
        _CACHE["last_results"] = res
    return out

